# revision 3
# baseline (speedup 1.0000x reference)
"""Distributed single-head transformer block on 8 TRN2 NeuronCores.

Collective-free restructuring. Algebraic folds done on the host
(weights only):
  - FFN has no activation between its two Linears, so it collapses to a
    single matrix Wf = W2@W1; the residual h folds in as Wg = Wf + I and
    LN0's gamma folds per-column: Wg2 = Wg * g0. The per-token LN0
    mean/rstd are applied as scalar corrections after one [D,D] matmul.
  - Q/K projections collapse into B = Wq.T @ Wk, so scores = x B x.T.
    Each core holds the FULL x (replicated at input-distribution time),
    so there is no K AllGather.
  - attn @ v = (P @ x) @ Wv.T + bv (softmax rows sum to 1), so there is
    no V AllGather either: P @ x uses the same resident full x.

All large matmuls run in fp8 DoubleRow (2 contraction k-tiles per
instruction, 157 TF/s). The dual-fp8 ldweights ISA check requires each
(2,128) weight pair-block to be contiguous in SBUF, so the host
pre-permutes every stationary operand into [..., 2, 128]-blocked layout;
moving operands are written [..., 2, TOK]-blocked on chip.

Scheduling structure (v2):
  - Input DMAs are issued at the top of the program across all three
    DGE queues (sync + scalar HWDGE, gpsimd SWDGE) so transfers start
    as soon as the BSP preamble ends; xTg8 is chunked so the scores
    phase starts as soon as its first chunk lands.
  - LN0 is folded via LN shift/scale invariance: LN1(acc) ==
    LN1(acc/rstd0), so the per-token correction becomes cfix2 =
    mu0*s2n + std0*cb -- a 2-row f32r matmul accumulated INTO the same
    PSUM tile as y. No rstd0 broadcast, no extra DVE passes.
  - The y phase + LN1 + writeback are split into two token halves,
    half-outer, so half 0's LN1 chain, normalize and output DMA overlap
    half 1's matmuls. Only half 1's epilogue is exposed at the end.

Per-core compute for its 512 tokens (T-domain, [feature, token]):
  xB^T   = B16 contract x^T      (fp8 DR, 16x-scaled for fp8 range)
  S^T_j  = x_full^T_j.T @ xB^T   (fp8 DR) -> exp(S/512) -> P fp8
  denom  = ones.T @ P            (fp8 DR ones-matmul)
  attnx  = x_full_j.T @ P^T      (fp8 DR), * 32/denom -> fp8
  attn^T = Wv16.T @ attnx        (fp8 DR), /512 + (x+bv) = res
  y^T    = Wg2^T.T @ res + cfix2 (bf16 + f32r 2-row fold)
  out    = rstd1*y - msr1*g1 + b1n   (per token-half)
"""

import numpy as np

P = 128
D = 1024
N = 4096
NCORES = 8
TOK = N // NCORES  # 512 tokens per core
HT = TOK // 2  # 256-token halves for the y/LN1/writeback pipeline
DK = D // P  # 8 feature tiles
KP = DK // 2  # 4 feature pair-tiles
NJ = N // P  # 32 global token tiles
JP = NJ // 2  # 16 token pair-tiles
EPS = 1e-5
WSCALE = 16.0  # fp8 range scale on B and Wv
ASCALE = 32.0  # fp8 range scale on normalized attnx
SINV = 1.0 / 512.0  # 1/(WSCALE*sqrt(D)) exp logit scale; also 1/(WSCALE*ASCALE)

_cache = {}


def _build_nc():
    import concourse.tile as tile
    from concourse import bacc, mybir
    from contextlib import ExitStack

    f32 = mybir.dt.float32
    f32r = mybir.dt.float32r
    bf16 = mybir.dt.bfloat16
    f8 = mybir.dt.float8e4
    Exp = mybir.ActivationFunctionType.Exp
    Sqrt = mybir.ActivationFunctionType.Sqrt
    Copy = mybir.ActivationFunctionType.Copy
    Square = mybir.ActivationFunctionType.Square
    Identity = mybir.ActivationFunctionType.Identity
    DR = mybir.MatmulPerfMode.DoubleRow
    mult = mybir.AluOpType.mult
    add = mybir.AluOpType.add

    nc = bacc.Bacc("TRN2", target_bir_lowering=False, debug=False, num_devices=NCORES)

    # local shard (T-layout, pre-blocked): bf16 copy carries +bv prefolded
    # (residual only); fp8 copy is pure x for the score path
    xTb = nc.dram_tensor("xTb", [P, DK, TOK], bf16, kind="ExternalInput").ap()
    xT8 = nc.dram_tensor("xT8", [P, KP, 2, TOK], f8, kind="ExternalInput").ap()
    # full x, both layouts, fp8, host pre-permuted into pair-blocked form
    xTg8 = nc.dram_tensor("xTg8", [P, NJ, KP, 2, P], f8, kind="ExternalInput").ap()
    xg8 = nc.dram_tensor("xg8", [P, DK, JP, 2, P], f8, kind="ExternalInput").ap()
    # folded weights (pair-blocked fp8 stationaries)
    B8d = nc.dram_tensor("B8d", [P, DK, KP, 2, P], f8, kind="ExternalInput").ap()
    Wv8 = nc.dram_tensor("Wv8", [P, DK, KP, 2, P], f8, kind="ExternalInput").ap()
    Wg2T = nc.dram_tensor("Wg2T", [P, DK, D], bf16, kind="ExternalInput").ap()
    # [g1; -b1n; 1/g1; 1/g1^2] merged, pre-blocked [P, 4, DK]
    lncon = nc.dram_tensor("lncon", [P, 4, DK], f32, kind="ExternalInput").ap()
    # cfix2 stationary rows [s2n; cb], blocked [2, DK, P]
    sc2 = nc.dram_tensor("sc2", [2, DK, P], f32, kind="ExternalInput").ap()
    outT = nc.dram_tensor("outT", [2, P, DK, HT], bf16, kind="ExternalOutput").ap()

    with tile.TileContext(nc) as tc, ExitStack() as ctx:
        ctx.enter_context(
            nc.allow_low_precision("f32r stat tiles are bit-identical fp32")
        )
        consts = ctx.enter_context(tc.tile_pool(name="consts", bufs=1))
        xin = ctx.enter_context(tc.tile_pool(name="xin", bufs=1))
        bigx = ctx.enter_context(tc.tile_pool(name="bigx", bufs=1))
        wp = ctx.enter_context(tc.tile_pool(name="wp", bufs=1))
        mid = ctx.enter_context(tc.tile_pool(name="mid", bufs=1))
        ev = ctx.enter_context(tc.tile_pool(name="ev", bufs=2))
        ps = ctx.enter_context(tc.tile_pool(name="ps", bufs=4, space="PSUM"))
        pss = ctx.enter_context(tc.tile_pool(name="pss", bufs=3, space="PSUM"))
        psb = ctx.enter_context(tc.tile_pool(name="psb", bufs=1, space="PSUM"))

        # ---- input DMA issue: everything up front, spread across the
        # three DGE queues. sync: xB-gating tensors then late xTg8
        # chunks; scalar: early xTg8 chunks (clear before the first exp);
        # gpsimd SWDGE: all background loads (needed only >35us in).
        xT8_sb = xin.tile([P, KP, 2, TOK], f8, tag="x8s")
        nc.sync.dma_start(out=xT8_sb, in_=xT8)
        B8_sb = wp.tile([P, DK, KP, 2, P], f8)
        for c in range(4):
            nc.sync.dma_start(
                out=B8_sb[:, 2 * c : 2 * c + 2], in_=B8d[:, 2 * c : 2 * c + 2]
            )
        xTg_sb = bigx.tile([P, NJ, KP, 2, P], f8)
        for c in range(4):  # scalar queue: chunks 0-3 (4 j-tiles each)
            nc.scalar.dma_start(
                out=xTg_sb[:, 4 * c : 4 * c + 4], in_=xTg8[:, 4 * c : 4 * c + 4]
            )
        for c in range(4, 8):  # sync queue: chunks 4-7
            nc.sync.dma_start(
                out=xTg_sb[:, 4 * c : 4 * c + 4], in_=xTg8[:, 4 * c : 4 * c + 4]
            )
        lncon_sb = consts.tile([P, 4, DK], f32)
        nc.gpsimd.dma_start(out=lncon_sb, in_=lncon)
        sc2_sb = consts.tile([2, DK, P], f32)
        nc.gpsimd.dma_start(out=sc2_sb, in_=sc2)
        xg_sb = bigx.tile([P, DK, JP, 2, P], f8)
        for c in range(8):
            nc.gpsimd.dma_start(out=xg_sb[:, c], in_=xg8[:, c])
        Wv8_sb = wp.tile([P, DK, KP, 2, P], f8)
        nc.gpsimd.dma_start(out=Wv8_sb, in_=Wv8)
        xTb_sb = xin.tile([P, DK, TOK], bf16)
        nc.gpsimd.dma_start(out=xTb_sb, in_=xTb)
        Wg2T_sb = wp.tile([P, DK, D], bf16)
        nc.gpsimd.dma_start(out=Wg2T_sb, in_=Wg2T)

        # ---- constants -------------------------------------------------
        ones8 = consts.tile([P, 2, 16], f8)
        nc.vector.memset(ones8, 1.0)
        ones_b = consts.tile([P, 1], bf16)
        nc.vector.memset(ones_b, 1.0)
        onesr_f32 = consts.tile([1, P], f32)
        nc.vector.memset(onesr_f32, 1.0)
        onesr = consts.tile([1, P], f32r)
        nc.vector.tensor_copy(onesr, onesr_f32)
        eps_sb = consts.tile([1, 1], f32)
        nc.vector.memset(eps_sb, EPS)
        g1_sb = lncon_sb[:, 0]
        nb1n_sb = lncon_sb[:, 1]
        invg_sb = consts.tile([P, 2, DK], bf16)
        nc.vector.tensor_copy(invg_sb, lncon_sb[:, 2:4])
        sc2r = consts.tile([2, DK, P], f32r)
        nc.vector.tensor_copy(sc2r, sc2_sb)
        # moving rows for the cfix2 fold: [mu0; std0] per token
        mvln = consts.tile([2, TOK], f32r)

        from concourse.bass import (
            AP,
            MemorySpace,
            assert_is_scalar,
            assert_partition_dims_match,
        )

        def act_raw(out, in_, func, bias=0.0, scale=1.0):
            eng = nc.scalar
            inputs = [eng.lower_ap(in_)]
            for arg in (bias, scale, 0.0):
                if isinstance(arg, AP):
                    assert_partition_dims_match(arg, in_)
                    assert_is_scalar(arg)
                    assert arg.space == MemorySpace.SBUF
                    inputs.append(eng.lower_ap(arg))
                else:
                    inputs.append(
                        mybir.ImmediateValue(dtype=mybir.dt.float32, value=arg)
                    )
            return eng.add_instruction(
                mybir.InstActivation(
                    name=eng.bass.get_next_instruction_name(),
                    func=func,
                    ins=inputs,
                    outs=[eng.lower_ap(out)],
                )
            )

        Rsqrt = mybir.ActivationFunctionType.Rsqrt
        Recip = mybir.ActivationFunctionType.Reciprocal

        _bc_n = [0]

        def bcast(row_f32r, tag, dt=f32, width=TOK):
            """[1, w] f32r -> [P, w] broadcast via PE outer product."""
            _bc_n[0] += 1
            pt = psb.tile([P, width], f32, tag="bc", name=f"bc_{_bc_n[0]}")
            nc.tensor.matmul(pt, onesr, row_f32r, start=True, stop=True)
            sb = consts.tile(
                [P, width], dt, name=f"bcs_{_bc_n[0]}", tag=f"bcs_{tag}"
            )
            nc.vector.tensor_copy(sb, pt)
            return sb

        # ---- xB = (16B) contract x (fp8 DoubleRow) ----------------------
        xB8_sb = mid.tile([P, KP, 2, TOK], f8)
        for m in range(DK):
            pt = ps.tile([P, TOK], f32, tag="pb")
            for k in range(KP):
                nc.tensor.matmul(
                    pt,
                    B8_sb[:, m, k],
                    xT8_sb[:, k],
                    start=(k == 0),
                    stop=(k == KP - 1),
                    perf_mode=DR,
                )
            nc.scalar.activation(xB8_sb[:, m // 2, m % 2, :], pt, Copy)

        # ---- scores S^T + exp -> fp8 probs, denominator interleaved ----
        pT8 = mid.tile([P, JP, 2, TOK], f8, tag="big16")
        psd = pss.tile([1, TOK], f32, tag="psm")
        for j in range(NJ):
            pt = ps.tile([P, TOK], f32, tag="pb")
            for k in range(KP):
                nc.tensor.matmul(
                    pt,
                    xTg_sb[:, j, k],
                    xB8_sb[:, k],
                    start=(k == 0),
                    stop=(k == KP - 1),
                    perf_mode=DR,
                )
            nc.scalar.activation(pT8[:, j // 2, j % 2, :], pt, Exp, bias=0.0, scale=SINV)
            if j % 2 == 1:
                nc.tensor.matmul(
                    psd,
                    ones8[:, :, 0:1],
                    pT8[:, j // 2],
                    start=(j == 1),
                    stop=(j == NJ - 1),
                    perf_mode=DR,
                )
        rden32 = consts.tile([1, TOK], f32r)
        act_raw(rden32, psd, Recip, bias=0.0, scale=1.0 / ASCALE)

        # ---- attnx = P @ x (fp8 DoubleRow), normalized to fp8. The rden
        # broadcast matmul is issued AFTER m=0's matmuls so the PE queue
        # doesn't head-of-line block on the scalar reciprocal chain. ----
        attnx8 = xin.tile([P, KP, 2, TOK], f8, tag="x8s", name="attnx8")
        rden_b = None
        for m in range(DK):
            pt = ps.tile([P, TOK], f32, tag="pb")
            for j in range(JP):
                nc.tensor.matmul(
                    pt,
                    xg_sb[:, m, j],
                    pT8[:, j],
                    start=(j == 0),
                    stop=(j == JP - 1),
                    perf_mode=DR,
                )
            if m == 0:
                rden_b = bcast(rden32, "rden")
            nc.vector.tensor_mul(attnx8[:, m // 2, m % 2, :], pt, rden_b)

        # ---- attn_out = attnx @ (16Wv).T / 512 + (x + bv) = res --------
        resb = xin.tile([P, DK, TOK], bf16)
        psm0 = pss.tile([1, TOK], f32, tag="psm")
        psq0 = pss.tile([1, TOK], f32, tag="psm")
        for m in range(DK):
            pt = ps.tile([P, TOK], f32, tag="pb")
            for k in range(KP):
                nc.tensor.matmul(
                    pt,
                    Wv8_sb[:, m, k],
                    attnx8[:, k],
                    start=(k == 0),
                    stop=(k == KP - 1),
                    perf_mode=DR,
                )
            t1 = ev.tile([P, TOK], f32, tag="sq")
            nc.scalar.activation(t1, pt, Copy, bias=0.0, scale=SINV)
            nc.vector.tensor_add(resb[:, m, :], t1, xTb_sb[:, m, :])
            sq = ev.tile([P, TOK], bf16, tag="sqb")
            nc.scalar.activation(sq, resb[:, m, :], Square)
            nc.tensor.matmul(
                psm0, ones_b, resb[:, m, :], start=(m == 0), stop=(m == DK - 1)
            )
            nc.tensor.matmul(psq0, ones_b, sq, start=(m == 0), stop=(m == DK - 1))

        # ---- LN0 scalars -> cfix2 moving rows [mu0; std0]. LN1 is
        # invariant to a per-token scale, so instead of multiplying y by
        # rstd0 we DIVIDE the correction by it: acc2 = y + mu0*s2n +
        # std0*cb, folded into the y PSUM group as one 2-row matmul. ----
        act_raw(mvln[0:1], psm0, Copy, bias=0.0, scale=1.0 / D)
        e20 = consts.tile([1, TOK], f32, tag="ln_e2")
        act_raw(e20, psq0, Copy, bias=0.0, scale=1.0 / D)
        mu20 = consts.tile([1, TOK], f32, tag="ln_mu2")
        nc.scalar.activation(mu20, mvln[0:1], Square)
        nc.vector.tensor_sub(e20, e20, mu20)
        std0 = consts.tile([1, TOK], f32r, tag="ln_std0")
        act_raw(std0, e20, Sqrt, bias=eps_sb[:])
        # engines can't write at partition base 1; hop via SBUF->SBUF DMA
        nc.scalar.dma_start(out=mvln[1:2], in_=std0)

        # ---- y = res @ Wg2.T (bf16) + cfix2, split into token halves
        # (half-outer) so half 0's LN1 chain + writeback + output DMA all
        # overlap half 1's matmuls. -------------------------------------
        acc = mid.tile([P, DK, TOK], bf16, tag="big16", name="acc")
        psm1 = [None, None]
        psq1 = [None, None]
        hss = [slice(0, HT), slice(HT, TOK)]

        def emit_y_half(h):
            hs = hss[h]
            psm1[h] = pss.tile([1, HT], f32, tag="psm", name=f"psm1{h}")
            psq1[h] = pss.tile([1, HT], f32, tag="psm", name=f"psq1{h}")
            lag = 2 if h == 0 else 0  # let the LN0 chain land before the
            pend = []  # first cfix2-fold closes a PSUM group
            for m in range(DK):
                pt = ps.tile([P, HT], f32, tag="pb", name=f"y{h}_{m}")
                for k in range(DK):
                    nc.tensor.matmul(
                        pt,
                        Wg2T_sb[:, k, m * P : (m + 1) * P],
                        resb[:, k, hs],
                        start=(k == 0),
                        stop=False,
                    )
                pend.append((m, pt))
                if len(pend) > lag:
                    _close_y(h, hs, *pend.pop(0))
            while pend:
                _close_y(h, hs, *pend.pop(0))

        def _close_y(h, hs, m, pt):
            nc.tensor.matmul(
                pt, sc2r[:, m], mvln[:, hs], start=False, stop=True
            )
            nc.vector.tensor_copy(acc[:, m, hs], pt)
            sq1 = ev.tile([P, HT], bf16, tag="sqb")
            nc.gpsimd.tensor_mul(sq1, acc[:, m, hs], acc[:, m, hs])
            nc.tensor.matmul(
                psm1[h],
                invg_sb[:, 0, m : m + 1],
                acc[:, m, hs],
                start=(m == 0),
                stop=(m == DK - 1),
            )
            nc.tensor.matmul(
                psq1[h],
                invg_sb[:, 1, m : m + 1],
                sq1,
                start=(m == 0),
                stop=(m == DK - 1),
            )

        def emit_ln1_wb(h):
            """LN1 chain + normalize + writeback for one token half."""
            hs = hss[h]
            mu1 = consts.tile([1, HT], f32r, tag="ln_mu", name=f"mu1{h}")
            act_raw(mu1, psm1[h], Copy, bias=0.0, scale=1.0 / D)
            e21 = consts.tile([1, HT], f32, tag="ln_e2", name=f"e21{h}")
            act_raw(e21, psq1[h], Copy, bias=0.0, scale=1.0 / D)
            mu21 = consts.tile([1, HT], f32, tag="ln_mu2", name=f"mu21{h}")
            nc.scalar.activation(mu21, mu1, Square)
            nc.vector.tensor_sub(e21, e21, mu21)
            rstd1 = consts.tile([1, HT], f32r, tag="ln_rstd", name=f"rstd1{h}")
            act_raw(rstd1, e21, Rsqrt, bias=eps_sb[:])
            rstd1_b = bcast(rstd1, "rstd1", bf16, width=HT)
            msr1 = consts.tile([1, HT], f32r, tag="ln_msr", name=f"msr1{h}")
            nc.vector.tensor_mul(msr1, mu1, rstd1)
            msr1_b = bcast(msr1, "msr1", bf16, width=HT)
            outh = mid.tile([P, DK, HT], bf16, tag="outh", bufs=2, name=f"outh{h}")
            for m in range(DK):
                c2 = ev.tile([P, HT], bf16, tag="ft1", bufs=3)
                nc.scalar.activation(
                    c2, msr1_b, Identity,
                    bias=nb1n_sb[:, m : m + 1],
                    scale=g1_sb[:, m : m + 1],
                )
                t1 = ev.tile([P, HT], bf16, tag="ot", bufs=3)
                nc.vector.tensor_mul(t1, acc[:, m, hs], rstd1_b)
                nc.vector.tensor_sub(outh[:, m], t1, c2)
            nc.sync.dma_start(out=outT[h], in_=outh)

        emit_y_half(0)
        emit_y_half(1)
        emit_ln1_wb(0)
        emit_ln1_wb(1)

    nc.finalize()
    return nc


def _get_nc():
    if "nc" not in _cache:
        _cache["nc"] = _build_nc()
    return _cache["nc"]


def _pair_block_m(w):
    """[D, M] -> [P, M//P, KP, 2, P] m-major pair-blocked stationary.

    w[d, m] with d = (2*k + i)*P + p, m = mt*P + c lands at
    out[p, mt, k, i, c] so each [2, P] block is contiguous and each
    output-tile's weights are one contiguous DRAM run per partition.
    """
    Dd, M = w.shape
    return np.ascontiguousarray(
        w.reshape(Dd // (2 * P), 2, P, M // P, P).transpose(2, 3, 0, 1, 4)
    )


def _tblock(w):
    """[D, M] -> [P, D//P, M]: d = k*P + p lands at [p, k, :]."""
    Dd, M = w.shape
    return np.ascontiguousarray(w.reshape(Dd // P, P, M).transpose(1, 0, 2))


def _make_in_maps(inputs):
    import ml_dtypes

    bf = ml_dtypes.bfloat16
    f8 = ml_dtypes.float8_e4m3

    x = np.asarray(inputs["x"], dtype=np.float64)
    Wq = np.asarray(inputs["Wq"], np.float64)
    Wk = np.asarray(inputs["Wk"], np.float64)
    Wv = np.asarray(inputs["Wv"], np.float64)
    W1 = np.asarray(inputs["W1"], np.float64)
    W2 = np.asarray(inputs["W2"], np.float64)
    g0 = np.asarray(inputs["g0"], np.float64)
    b0 = np.asarray(inputs["b0"], np.float64)
    b1 = np.asarray(inputs["b1"], np.float64)
    b2 = np.asarray(inputs["b2"], np.float64)

    xf32 = x.astype(np.float32)
    x8 = xf32.astype(f8)
    xT8f = np.ascontiguousarray(xf32.T).astype(f8)

    Wf = W2 @ W1
    Wg = Wf + np.eye(D)
    g1f = np.asarray(inputs["g1"], np.float64)
    Wg2 = Wg * g0[None, :] * g1f[:, None]
    lncon = np.stack(
        [
            g1f.astype(np.float32),
            (-np.asarray(inputs["b1n"], np.float64)).astype(np.float32),
            (1.0 / g1f).astype(np.float32),
            (1.0 / (g1f * g1f)).astype(np.float32),
        ],
        axis=0,
    )  # [4, D]
    sc2 = np.stack(
        [
            (-Wg2.sum(axis=1)).astype(np.float32),
            ((Wg @ b0 + W2 @ b1 + b2) * g1f).astype(np.float32),
        ],
        axis=0,
    )  # [2, D]
    shared = {
        "B8d": _pair_block_m((WSCALE * (Wq.T @ Wk)).astype(np.float32).astype(f8)),
        "Wv8": _pair_block_m((WSCALE * Wv.T).astype(np.float32).astype(f8)),
        "Wg2T": _tblock(Wg2.T.astype(np.float32).astype(bf)),
        # [P, 4, DK]: row d = m*P + p of each vector at [p, i, m]
        "lncon": np.ascontiguousarray(
            lncon.reshape(4, DK, P).transpose(2, 0, 1)
        ),
        # [2, DK, P]: row i, d = m*P + c at [i, m, c]
        "sc2": np.ascontiguousarray(sc2.reshape(2, DK, P)),
        # scores stationary: [p, jt, k, i, m] = x[jt*P+m, (2k+i)*P+p]
        "xTg8": np.ascontiguousarray(
            xT8f.reshape(KP, 2, P, NJ, P).transpose(2, 3, 0, 1, 4)
        ),
        # attnx stationary: [p, mt, jp, i, m] = x[(2jp+i)*P+p, mt*P+m]
        "xg8": np.ascontiguousarray(
            x8.reshape(JP, 2, P, DK, P).transpose(2, 3, 0, 1, 4)
        ),
    }
    bvf = np.asarray(inputs["bv"], np.float64)
    xTbv = (x + bvf[None, :]).T.astype(np.float32)
    xT = np.ascontiguousarray(xf32.T)
    in_maps = []
    for c in range(NCORES):
        m = dict(shared)
        m["xTb"] = _tblock(
            np.ascontiguousarray(xTbv[:, c * TOK : (c + 1) * TOK]).astype(bf)
        )
        # moving operand of xB: [p, k, i, t] = x[t, (2k+i)*P+p]
        xTl = np.ascontiguousarray(xT[:, c * TOK : (c + 1) * TOK]).astype(f8)
        m["xT8"] = np.ascontiguousarray(
            xTl.reshape(KP, 2, P, TOK).transpose(2, 0, 1, 3)
        )
        in_maps.append(m)
    return in_maps


def _assemble(res):
    out = np.empty((N, D), dtype=np.float32)
    for c in range(NCORES):
        # outT [2, P, DK, HT] bf16: out[h*HT+t, m*P+p] = arr[h, p, m, t]
        arr = np.asarray(res.results[c]["outT"], dtype=np.float32)
        out[c * TOK : (c + 1) * TOK, :] = arr.transpose(0, 3, 2, 1).reshape(TOK, D)
    return out


def kernel(**inputs):
    from concourse import bass_utils

    nc = _get_nc()
    res = bass_utils.run_bass_kernel_spmd(
        nc, _make_in_maps(inputs), core_ids=list(range(NCORES)), trace=False
    )
    return _assemble(res)


def run_traced(inputs):
    """Like kernel() but with NTFF tracing; returns (out, exec_time_ns, results)."""
    import hookshim

    hookshim.install()
    from concourse import bass_utils

    nc = _get_nc()
    res = bass_utils.run_bass_kernel_spmd(
        nc, _make_in_maps(inputs), core_ids=list(range(NCORES)), trace=True
    )
    return _assemble(res), res.exec_time_ns, res


# revision 5
# speedup vs baseline: 1.0254x; 1.0254x over previous
"""Distributed single-head transformer block on 8 TRN2 NeuronCores.

Collective-free restructuring. Algebraic folds done on the host
(weights only):
  - FFN has no activation between its two Linears, so it collapses to a
    single matrix Wf = W2@W1; the residual h folds in as Wg = Wf + I and
    LN0's gamma folds per-column: Wg2 = Wg * g0.
  - Q/K projections collapse into B = Wq.T @ Wk, so scores = x B x.T.
    Each core holds the FULL x (replicated at input-distribution time),
    so there is no K AllGather.
  - attn @ v = (P @ x) @ Wv.T + bv (softmax rows sum to 1), so there is
    no V AllGather either: P @ x uses the same resident full x.
  - LN0 is folded via LN shift/scale invariance: LN1(acc) ==
    LN1(acc/rstd0), so the per-token LN0 correction becomes
    acc2 = y + mu0*s2n (+ std0*cb when biases exist) -- 1-row bf16
    matmuls accumulated INTO the same PSUM group as y. No rstd0
    broadcast, and with zero biases the whole LN0 variance path
    (psq0 / Square / Sqrt) vanishes.

All large matmuls run in fp8 DoubleRow (2 contraction k-tiles per
instruction, 157 TF/s). The dual-fp8 ldweights ISA check requires each
(2,128) weight pair-block to be contiguous in SBUF, so the host
pre-permutes every stationary operand into [..., 2, 128]-blocked layout;
moving operands are written [..., 2, TOK]-blocked on chip.

Scheduling structure (v3):
  - Input DMAs all issue at the top across the two HWDGE rings, with
    background tensors queued BEHIND the front-critical ones on the
    same in-order rings so they cannot steal front bandwidth. xT8+B8d
    ride the scalar ring (observed to start ~1.5us earlier), xTg8 is
    chunked so scores start on the first chunk.
  - The y phase + LN1 + writeback are split into two token halves,
    half-outer; half 0's epilogue is EMITTED inside half 1's m-loop so
    the in-order engine queues interleave it under half 1's matmuls.
  - Engine assignment in the y window: PE matmuls+fold, scalar evac
    (psum->bf16) + LN chains + most c2, DVE sq + writeback mul/sub,
    gpsimd c2+writeback for 2 tiles per half.

Per-core compute for its 512 tokens (T-domain, [feature, token]):
  xB^T   = B16 contract x^T      (fp8 DR, 16x-scaled for fp8 range)
  S^T_j  = x_full^T_j.T @ xB^T   (fp8 DR) -> exp(S/512) -> P fp8
  denom  = ones.T @ P            (fp8 DR ones-matmul)
  attnx  = x_full_j.T @ P^T      (fp8 DR), * 32/denom -> fp8
  attn^T = Wv16.T @ attnx        (fp8 DR), /512 + (x+bv) = res
  y^T    = Wg2^T.T @ res + mu0*s2n  (bf16, LN0 folded)
  out    = rstd1*y - msr1*g1 + b1n   (per token-half)
"""

import numpy as np

P = 128
D = 1024
N = 4096
NCORES = 8
TOK = N // NCORES  # 512 tokens per core
HT = TOK // 2  # 256-token halves for the y/LN1/writeback pipeline
DK = D // P  # 8 feature tiles
KP = DK // 2  # 4 feature pair-tiles
NJ = N // P  # 32 global token tiles
JP = NJ // 2  # 16 token pair-tiles
EPS = 1e-5
WSCALE = 16.0  # fp8 range scale on B and Wv
ASCALE = 32.0  # fp8 range scale on normalized attnx
SINV = 1.0 / 512.0  # 1/(WSCALE*sqrt(D)) exp logit scale; also 1/(WSCALE*ASCALE)

_cache = {}


def _build_nc(has_cb):
    import concourse.tile as tile
    from concourse import bacc, mybir
    from contextlib import ExitStack

    f32 = mybir.dt.float32
    bf16 = mybir.dt.bfloat16
    f8 = mybir.dt.float8e4
    Exp = mybir.ActivationFunctionType.Exp
    Sqrt = mybir.ActivationFunctionType.Sqrt
    Copy = mybir.ActivationFunctionType.Copy
    Square = mybir.ActivationFunctionType.Square
    Identity = mybir.ActivationFunctionType.Identity
    DR = mybir.MatmulPerfMode.DoubleRow
    mult = mybir.AluOpType.mult
    add = mybir.AluOpType.add

    nc = bacc.Bacc("TRN2", target_bir_lowering=False, debug=False, num_devices=NCORES)

    # local shard (T-layout, pre-blocked): bf16 copy carries +bv prefolded
    # (residual only); fp8 copy is pure x for the score path
    xTb = nc.dram_tensor("xTb", [P, DK, TOK], bf16, kind="ExternalInput").ap()
    xT8 = nc.dram_tensor("xT8", [P, KP, 2, TOK], f8, kind="ExternalInput").ap()
    # full x, both layouts, fp8, host pre-permuted into pair-blocked form
    xTg8 = nc.dram_tensor("xTg8", [P, NJ, KP, 2, P], f8, kind="ExternalInput").ap()
    xg8 = nc.dram_tensor("xg8", [P, DK, JP, 2, P], f8, kind="ExternalInput").ap()
    # folded weights (pair-blocked fp8 stationaries)
    B8d = nc.dram_tensor("B8d", [P, DK, KP, 2, P], f8, kind="ExternalInput").ap()
    Wv8 = nc.dram_tensor("Wv8", [P, DK, KP, 2, P], f8, kind="ExternalInput").ap()
    Wg2T = nc.dram_tensor("Wg2T", [P, DK, D], bf16, kind="ExternalInput").ap()
    # [g1; -b1n; 1/g1; 1/g1^2] merged, pre-blocked [P, 4, DK]
    lncon = nc.dram_tensor("lncon", [P, 4, DK], f32, kind="ExternalInput").ap()
    # cfix fold stationaries, 1-row blocked [1, DK, P] bf16
    s2nr = nc.dram_tensor("s2nr", [1, DK, P], bf16, kind="ExternalInput").ap()
    cbr = (
        nc.dram_tensor("cbr", [1, DK, P], bf16, kind="ExternalInput").ap()
        if has_cb
        else None
    )
    outT = nc.dram_tensor("outT", [2, P, DK, HT], bf16, kind="ExternalOutput").ap()

    with tile.TileContext(nc) as tc, ExitStack() as ctx:
        ctx.enter_context(
            nc.allow_low_precision("bf16 stat rows; LN-invariant rescale")
        )
        consts = ctx.enter_context(tc.tile_pool(name="consts", bufs=1))
        xin = ctx.enter_context(tc.tile_pool(name="xin", bufs=1))
        bigx = ctx.enter_context(tc.tile_pool(name="bigx", bufs=1))
        wp = ctx.enter_context(tc.tile_pool(name="wp", bufs=1))
        mid = ctx.enter_context(tc.tile_pool(name="mid", bufs=1))
        ev = ctx.enter_context(tc.tile_pool(name="ev", bufs=2))
        ps = ctx.enter_context(tc.tile_pool(name="ps", bufs=4, space="PSUM"))
        pss = ctx.enter_context(tc.tile_pool(name="pss", bufs=3, space="PSUM"))
        psb = ctx.enter_context(tc.tile_pool(name="psb", bufs=1, space="PSUM"))

        # ---- input DMA issue: all up front on the two in-order HWDGE
        # rings; background tensors queue BEHIND front-critical ones.
        # scalar ring: xT8 -> B8d (gates xB) -> late xTg8 -> small consts
        # sync ring: early xTg8 (gates scores) -> xg8 -> Wv8/xTb/Wg2T
        xT8_sb = xin.tile([P, KP, 2, TOK], f8, tag="x8s")
        nc.scalar.dma_start(out=xT8_sb, in_=xT8)
        B8_sb = wp.tile([P, DK, KP, 2, P], f8)
        for c in range(4):
            nc.scalar.dma_start(
                out=B8_sb[:, 2 * c : 2 * c + 2], in_=B8d[:, 2 * c : 2 * c + 2]
            )
        xTg_sb = bigx.tile([P, NJ, KP, 2, P], f8)
        for c in range(4):  # sync ring: chunks 0-3 (4 j-tiles each)
            nc.sync.dma_start(
                out=xTg_sb[:, 4 * c : 4 * c + 4], in_=xTg8[:, 4 * c : 4 * c + 4]
            )
        for c in range(4, 8):  # scalar ring: chunks 4-7
            nc.scalar.dma_start(
                out=xTg_sb[:, 4 * c : 4 * c + 4], in_=xTg8[:, 4 * c : 4 * c + 4]
            )
        xg_sb = bigx.tile([P, DK, JP, 2, P], f8)
        for c in range(6):
            nc.sync.dma_start(out=xg_sb[:, c], in_=xg8[:, c])
        for c in range(6, 8):
            nc.scalar.dma_start(out=xg_sb[:, c], in_=xg8[:, c])
        Wv8_sb = wp.tile([P, DK, KP, 2, P], f8)
        nc.sync.dma_start(out=Wv8_sb, in_=Wv8)
        xTb_sb = xin.tile([P, DK, TOK], bf16)
        nc.sync.dma_start(out=xTb_sb, in_=xTb)
        Wg2T_sb = wp.tile([P, DK, D], bf16)
        nc.sync.dma_start(out=Wg2T_sb, in_=Wg2T)
        lncon_sb = consts.tile([P, 4, DK], f32)
        nc.scalar.dma_start(out=lncon_sb, in_=lncon)
        s2n_sb = consts.tile([1, DK, P], bf16)
        nc.scalar.dma_start(out=s2n_sb, in_=s2nr)
        if has_cb:
            cb_sb = consts.tile([1, DK, P], bf16)
            nc.scalar.dma_start(out=cb_sb, in_=cbr)

        # ---- constants -------------------------------------------------
        ones8 = consts.tile([P, 2, 16], f8)
        nc.vector.memset(ones8, 1.0)
        ones_b = consts.tile([P, 1], bf16)
        nc.vector.memset(ones_b, 1.0)
        onesr = consts.tile([1, P], bf16)
        nc.vector.memset(onesr, 1.0)
        eps_sb = consts.tile([1, 1], f32)
        nc.vector.memset(eps_sb, EPS)
        g1_sb = lncon_sb[:, 0]
        nb1n_sb = lncon_sb[:, 1]
        invg_sb = consts.tile([P, 2, DK], bf16)
        nc.vector.tensor_copy(invg_sb, lncon_sb[:, 2:4])

        from concourse.bass import (
            AP,
            MemorySpace,
            assert_is_scalar,
            assert_partition_dims_match,
        )

        def act_raw(out, in_, func, bias=0.0, scale=1.0):
            eng = nc.scalar
            inputs = [eng.lower_ap(in_)]
            for arg in (bias, scale, 0.0):
                if isinstance(arg, AP):
                    assert_partition_dims_match(arg, in_)
                    assert_is_scalar(arg)
                    assert arg.space == MemorySpace.SBUF
                    inputs.append(eng.lower_ap(arg))
                else:
                    inputs.append(
                        mybir.ImmediateValue(dtype=mybir.dt.float32, value=arg)
                    )
            return eng.add_instruction(
                mybir.InstActivation(
                    name=eng.bass.get_next_instruction_name(),
                    func=func,
                    ins=inputs,
                    outs=[eng.lower_ap(out)],
                )
            )

        Rsqrt = mybir.ActivationFunctionType.Rsqrt
        Recip = mybir.ActivationFunctionType.Reciprocal

        _bc_n = [0]

        def bcast(row_b, tag, width=TOK):
            """[1, w] bf16 -> [P, w] bf16 broadcast via PE outer product."""
            _bc_n[0] += 1
            pt = psb.tile([P, width], f32, tag="bc", name=f"bc_{_bc_n[0]}")
            nc.tensor.matmul(pt, onesr, row_b, start=True, stop=True)
            sb = consts.tile(
                [P, width], bf16, name=f"bcs_{_bc_n[0]}", tag=f"bcs_{tag}"
            )
            nc.vector.tensor_copy(sb, pt)
            return sb

        # ---- xB = (16B) contract x (fp8 DoubleRow) ----------------------
        xB8_sb = mid.tile([P, KP, 2, TOK], f8)
        for m in range(DK):
            pt = ps.tile([P, TOK], f32, tag="pb")
            for k in range(KP):
                nc.tensor.matmul(
                    pt,
                    B8_sb[:, m, k],
                    xT8_sb[:, k],
                    start=(k == 0),
                    stop=(k == KP - 1),
                    perf_mode=DR,
                )
            nc.scalar.activation(xB8_sb[:, m // 2, m % 2, :], pt, Copy)

        # ---- scores S^T + exp -> fp8 probs, denominator interleaved ----
        pT8 = mid.tile([P, JP, 2, TOK], f8, tag="big16")
        psd = pss.tile([1, TOK], f32, tag="psm")
        for j in range(NJ):
            pt = ps.tile([P, TOK], f32, tag="pb")
            for k in range(KP):
                nc.tensor.matmul(
                    pt,
                    xTg_sb[:, j, k],
                    xB8_sb[:, k],
                    start=(k == 0),
                    stop=(k == KP - 1),
                    perf_mode=DR,
                )
            nc.scalar.activation(pT8[:, j // 2, j % 2, :], pt, Exp, bias=0.0, scale=SINV)
            if j % 2 == 1:
                nc.tensor.matmul(
                    psd,
                    ones8[:, :, 0:1],
                    pT8[:, j // 2],
                    start=(j == 1),
                    stop=(j == NJ - 1),
                    perf_mode=DR,
                )
        rden32 = consts.tile([1, TOK], bf16)
        act_raw(rden32, psd, Recip, bias=0.0, scale=1.0 / ASCALE)

        # ---- attnx = P @ x (fp8 DoubleRow), normalized to fp8. The rden
        # broadcast matmul is issued AFTER m=0's matmuls so the PE queue
        # doesn't head-of-line block on the scalar reciprocal chain. ----
        attnx8 = xin.tile([P, KP, 2, TOK], f8, tag="x8s", name="attnx8")
        rden_b = None
        for m in range(DK):
            pt = ps.tile([P, TOK], f32, tag="pb")
            for j in range(JP):
                nc.tensor.matmul(
                    pt,
                    xg_sb[:, m, j],
                    pT8[:, j],
                    start=(j == 0),
                    stop=(j == JP - 1),
                    perf_mode=DR,
                )
            if m == 0:
                rden_b = bcast(rden32, "rden")
            nc.vector.tensor_mul(attnx8[:, m // 2, m % 2, :], pt, rden_b)

        # ---- attn_out = attnx @ (16Wv).T / 512 + (x + bv) = res --------
        resb = xin.tile([P, DK, TOK], bf16)
        psm0 = pss.tile([1, TOK], f32, tag="psm")
        psq0 = pss.tile([1, TOK], f32, tag="psm") if has_cb else None
        for m in range(DK):
            pt = ps.tile([P, TOK], f32, tag="pb")
            for k in range(KP):
                nc.tensor.matmul(
                    pt,
                    Wv8_sb[:, m, k],
                    attnx8[:, k],
                    start=(k == 0),
                    stop=(k == KP - 1),
                    perf_mode=DR,
                )
            t1 = ev.tile([P, TOK], f32, tag="sq")
            nc.scalar.activation(t1, pt, Copy, bias=0.0, scale=SINV)
            nc.vector.tensor_add(resb[:, m, :], t1, xTb_sb[:, m, :])
            nc.tensor.matmul(
                psm0, ones_b, resb[:, m, :], start=(m == 0), stop=(m == DK - 1)
            )
            if has_cb:
                sq = ev.tile([P, TOK], bf16, tag="sqb")
                nc.scalar.activation(sq, resb[:, m, :], Square)
                nc.tensor.matmul(
                    psq0, ones_b, sq, start=(m == 0), stop=(m == DK - 1)
                )

        # ---- LN0 scalars for the cfix fold. LN1 is invariant to a
        # per-token scale, so instead of multiplying y by rstd0 we DIVIDE
        # the correction by it: acc2 = y + mu0*s2n (+ std0*cb). --------
        mu0b = consts.tile([1, TOK], bf16, tag="ln_mu0")
        act_raw(mu0b, psm0, Copy, bias=0.0, scale=1.0 / D)
        if has_cb:
            e20 = consts.tile([1, TOK], f32, tag="ln_e2")
            act_raw(e20, psq0, Copy, bias=0.0, scale=1.0 / D)
            mu0f = consts.tile([1, TOK], f32, tag="ln_mu0f")
            act_raw(mu0f, psm0, Copy, bias=0.0, scale=1.0 / D)
            mu20 = consts.tile([1, TOK], f32, tag="ln_mu2")
            nc.scalar.activation(mu20, mu0f, Square)
            nc.vector.tensor_sub(e20, e20, mu20)
            std0b = consts.tile([1, TOK], bf16, tag="ln_std0")
            act_raw(std0b, e20, Sqrt, bias=eps_sb[:])

        # ---- y = res @ Wg2.T (bf16) + cfix fold, split into token
        # halves (half-outer); half 0's LN1 chain + writeback + output
        # DMA are emitted INSIDE half 1's m-loop so the in-order engine
        # queues overlap them with half 1's matmuls. ---------------------
        acc = mid.tile([P, DK, TOK], bf16, tag="big16", name="acc")
        psm1 = [None, None]
        psq1 = [None, None]
        hss = [slice(0, HT), slice(HT, TOK)]

        def emit_y_half(h, hook_m=None, hook=None):
            hs = hss[h]
            psm1[h] = pss.tile([1, HT], f32, tag="psm", name=f"psm1{h}")
            psq1[h] = pss.tile([1, HT], f32, tag="psm", name=f"psq1{h}")
            lag = 2 if h == 0 else 0  # let mu0 land before the first
            pend = []  # cfix fold closes a PSUM group
            for m in range(DK):
                pt = ps.tile([P, HT], f32, tag="pb", name=f"y{h}_{m}")
                for k in range(DK):
                    nc.tensor.matmul(
                        pt,
                        Wg2T_sb[:, k, m * P : (m + 1) * P],
                        resb[:, k, hs],
                        start=(k == 0),
                        stop=False,
                    )
                pend.append((m, pt))
                if len(pend) > lag:
                    _close_y(h, hs, *pend.pop(0))
                if m == hook_m and hook is not None:
                    hook()
            while pend:
                _close_y(h, hs, *pend.pop(0))

        def _close_y(h, hs, m, pt):
            nc.tensor.matmul(
                pt, s2n_sb[:, m], mu0b[0:1, hs], start=False,
                stop=not has_cb,
            )
            if has_cb:
                nc.tensor.matmul(
                    pt, cb_sb[:, m], std0b[0:1, hs], start=False, stop=True
                )
            nc.scalar.activation(acc[:, m, hs], pt, Copy)
            sq1 = ev.tile([P, HT], bf16, tag="sqb")
            nc.vector.tensor_mul(sq1, acc[:, m, hs], acc[:, m, hs])
            nc.tensor.matmul(
                psm1[h],
                invg_sb[:, 0, m : m + 1],
                acc[:, m, hs],
                start=(m == 0),
                stop=(m == DK - 1),
            )
            nc.tensor.matmul(
                psq1[h],
                invg_sb[:, 1, m : m + 1],
                sq1,
                start=(m == 0),
                stop=(m == DK - 1),
            )

        def emit_ln1_wb(h):
            """LN1 chain + normalize + writeback for one token half."""
            hs = hss[h]
            mu1 = consts.tile([1, HT], f32, tag="ln_mu", name=f"mu1{h}")
            act_raw(mu1, psm1[h], Copy, bias=0.0, scale=1.0 / D)
            e21 = consts.tile([1, HT], f32, tag="ln_e2b", name=f"e21{h}")
            act_raw(e21, psq1[h], Copy, bias=0.0, scale=1.0 / D)
            mu21 = consts.tile([1, HT], f32, tag="ln_mu2b", name=f"mu21{h}")
            nc.scalar.activation(mu21, mu1, Square)
            nc.vector.tensor_sub(e21, e21, mu21)
            rstd1 = consts.tile([1, HT], bf16, tag="ln_rstd", name=f"rstd1{h}")
            act_raw(rstd1, e21, Rsqrt, bias=eps_sb[:])
            rstd1_b = bcast(rstd1, "rstd1", width=HT)
            msr1 = consts.tile([1, HT], bf16, tag="ln_msr", name=f"msr1{h}")
            nc.vector.tensor_mul(msr1, mu1, rstd1)
            msr1_b = bcast(msr1, "msr1", width=HT)
            outh = mid.tile([P, DK, HT], bf16, tag="outh", bufs=2, name=f"outh{h}")
            for m in range(DK):
                gp = m % 4 == 3  # gpsimd owns 2 tiles per half end-to-end
                c2 = ev.tile([P, HT], bf16, tag="ft1", bufs=3)
                if gp:
                    nc.gpsimd.tensor_scalar(
                        c2, msr1_b,
                        g1_sb[:, m : m + 1], nb1n_sb[:, m : m + 1],
                        op0=mult, op1=add,
                    )
                else:
                    nc.scalar.activation(
                        c2, msr1_b, Identity,
                        bias=nb1n_sb[:, m : m + 1],
                        scale=g1_sb[:, m : m + 1],
                    )
                eng = nc.gpsimd if gp else nc.vector
                t1 = ev.tile([P, HT], bf16, tag="ot", bufs=3)
                eng.tensor_mul(t1, acc[:, m, hs], rstd1_b)
                eng.tensor_sub(outh[:, m], t1, c2)
                if m == 3:
                    nc.sync.dma_start(out=outT[h, :, 0:4], in_=outh[:, 0:4])
            nc.sync.dma_start(out=outT[h, :, 4:8], in_=outh[:, 4:8])

        emit_y_half(0)
        emit_y_half(1, hook_m=1, hook=lambda: emit_ln1_wb(0))
        emit_ln1_wb(1)

    nc.finalize()
    return nc


def _get_nc(has_cb):
    key = ("nc", has_cb)
    if key not in _cache:
        _cache[key] = _build_nc(has_cb)
    return _cache[key]


def _pair_block_m(w):
    """[D, M] -> [P, M//P, KP, 2, P] m-major pair-blocked stationary.

    w[d, m] with d = (2*k + i)*P + p, m = mt*P + c lands at
    out[p, mt, k, i, c] so each [2, P] block is contiguous and each
    output-tile's weights are one contiguous DRAM run per partition.
    """
    Dd, M = w.shape
    return np.ascontiguousarray(
        w.reshape(Dd // (2 * P), 2, P, M // P, P).transpose(2, 3, 0, 1, 4)
    )


def _tblock(w):
    """[D, M] -> [P, D//P, M]: d = k*P + p lands at [p, k, :]."""
    Dd, M = w.shape
    return np.ascontiguousarray(w.reshape(Dd // P, P, M).transpose(1, 0, 2))


def _make_in_maps(inputs):
    import ml_dtypes

    bf = ml_dtypes.bfloat16
    f8 = ml_dtypes.float8_e4m3

    x = np.asarray(inputs["x"], dtype=np.float64)
    Wq = np.asarray(inputs["Wq"], np.float64)
    Wk = np.asarray(inputs["Wk"], np.float64)
    Wv = np.asarray(inputs["Wv"], np.float64)
    W1 = np.asarray(inputs["W1"], np.float64)
    W2 = np.asarray(inputs["W2"], np.float64)
    g0 = np.asarray(inputs["g0"], np.float64)
    b0 = np.asarray(inputs["b0"], np.float64)
    b1 = np.asarray(inputs["b1"], np.float64)
    b2 = np.asarray(inputs["b2"], np.float64)

    xf32 = x.astype(np.float32)
    x8 = xf32.astype(f8)
    xT8f = np.ascontiguousarray(xf32.T).astype(f8)

    Wf = W2 @ W1
    Wg = Wf + np.eye(D)
    g1f = np.asarray(inputs["g1"], np.float64)
    Wg2 = Wg * g0[None, :] * g1f[:, None]
    lncon = np.stack(
        [
            g1f.astype(np.float32),
            (-np.asarray(inputs["b1n"], np.float64)).astype(np.float32),
            (1.0 / g1f).astype(np.float32),
            (1.0 / (g1f * g1f)).astype(np.float32),
        ],
        axis=0,
    )  # [4, D]
    s2n = (-Wg2.sum(axis=1)).astype(np.float32)
    cb = ((Wg @ b0 + W2 @ b1 + b2) * g1f).astype(np.float32)
    shared = {
        "B8d": _pair_block_m((WSCALE * (Wq.T @ Wk)).astype(np.float32).astype(f8)),
        "Wv8": _pair_block_m((WSCALE * Wv.T).astype(np.float32).astype(f8)),
        "Wg2T": _tblock(Wg2.T.astype(np.float32).astype(bf)),
        # [P, 4, DK]: row d = m*P + p of each vector at [p, i, m]
        "lncon": np.ascontiguousarray(
            lncon.reshape(4, DK, P).transpose(2, 0, 1)
        ),
        "s2nr": np.ascontiguousarray(s2n.reshape(1, DK, P)).astype(bf),
        "cbr": np.ascontiguousarray(cb.reshape(1, DK, P)).astype(bf),
        # scores stationary: [p, jt, k, i, m] = x[jt*P+m, (2k+i)*P+p]
        "xTg8": np.ascontiguousarray(
            xT8f.reshape(KP, 2, P, NJ, P).transpose(2, 3, 0, 1, 4)
        ),
        # attnx stationary: [p, mt, jp, i, m] = x[(2jp+i)*P+p, mt*P+m]
        "xg8": np.ascontiguousarray(
            x8.reshape(JP, 2, P, DK, P).transpose(2, 3, 0, 1, 4)
        ),
    }
    has_cb = bool(np.any(cb != 0.0))
    bvf = np.asarray(inputs["bv"], np.float64)
    xTbv = (x + bvf[None, :]).T.astype(np.float32)
    xT = np.ascontiguousarray(xf32.T)
    in_maps = []
    for c in range(NCORES):
        m = dict(shared)
        m["xTb"] = _tblock(
            np.ascontiguousarray(xTbv[:, c * TOK : (c + 1) * TOK]).astype(bf)
        )
        # moving operand of xB: [p, k, i, t] = x[t, (2k+i)*P+p]
        xTl = np.ascontiguousarray(xT[:, c * TOK : (c + 1) * TOK]).astype(f8)
        m["xT8"] = np.ascontiguousarray(
            xTl.reshape(KP, 2, P, TOK).transpose(2, 0, 1, 3)
        )
        if not has_cb:
            del m["cbr"]
        in_maps.append(m)
    return in_maps, has_cb


def _assemble(res):
    out = np.empty((N, D), dtype=np.float32)
    for c in range(NCORES):
        # outT [2, P, DK, HT] bf16: out[h*HT+t, m*P+p] = arr[h, p, m, t]
        arr = np.asarray(res.results[c]["outT"], dtype=np.float32)
        out[c * TOK : (c + 1) * TOK, :] = arr.transpose(0, 3, 2, 1).reshape(TOK, D)
    return out


def kernel(**inputs):
    from concourse import bass_utils

    in_maps, has_cb = _make_in_maps(inputs)
    nc = _get_nc(has_cb)
    res = bass_utils.run_bass_kernel_spmd(
        nc, in_maps, core_ids=list(range(NCORES)), trace=False
    )
    return _assemble(res)


def run_traced(inputs):
    """Like kernel() but with NTFF tracing; returns (out, exec_time_ns, results)."""
    import hookshim

    hookshim.install()
    from concourse import bass_utils

    in_maps, has_cb = _make_in_maps(inputs)
    nc = _get_nc(has_cb)
    res = bass_utils.run_bass_kernel_spmd(
        nc, in_maps, core_ids=list(range(NCORES)), trace=True
    )
    return _assemble(res), res.exec_time_ns, res


# revision 7
# speedup vs baseline: 1.1169x; 1.0892x over previous
"""Distributed single-head transformer block on 8 TRN2 NeuronCores.

Collective-free restructuring. Algebraic folds done on the host
(weights only):
  - FFN has no activation between its two Linears, so it collapses to a
    single matrix Wf = W2@W1; the residual h folds in as Wg = Wf + I and
    LN0's gamma folds per-column: Wg2 = Wg * g0.
  - Q/K projections collapse into B = Wq.T @ Wk, so scores = x B x.T.
    Each core holds the FULL x (replicated at input-distribution time),
    so there is no K AllGather.
  - attn @ v = (P @ x) @ Wv.T + bv (softmax rows sum to 1), so there is
    no V AllGather either: P @ x uses the same resident full x.
  - LN0 is folded via LN shift/scale invariance: LN1(acc) ==
    LN1(acc/rstd0), so the per-token LN0 correction becomes
    acc2 = y + mu0*s2n (+ std0*cb when biases exist) -- 1-row bf16
    matmuls accumulated INTO the same PSUM group as y. With zero biases
    the whole LN0 variance path vanishes.
  - LN1 writeback uses out = rstd1*(acc - c3), c3 = mu1 (x) g1 +
    std1 (x) (-b1n): c3 rides PE 1-row outer products that need only
    mu1/std1 (ready BEFORE rstd1), so the scalar engine's c2 chain is
    off the tail's critical path.

All large matmuls run in fp8 DoubleRow (2 contraction k-tiles per
instruction, 157 TF/s). The dual-fp8 ldweights ISA check requires each
(2,128) weight pair-block to be contiguous in SBUF, so the host
pre-permutes every stationary operand into [..., 2, 128]-blocked layout;
moving operands are written [..., 2, TOK]-blocked on chip.

Scheduling (v4):
  - sync HWDGE ring (in-order): xT8 -> B8d -> xTg8 in 8 chunks, so xB
    starts on the first kilobytes and scores start on the first chunk.
    The scalar ring carries NO early DMAs (its engine runs the evac/exp
    path). Background tensors ride gpsimd SWDGE, EMITTED after the xB
    phase so the in-order gpsimd queue naturally delays their kicks
    behind a tiny gate-copy of xB8 -- they cannot steal front bandwidth.
  - y + LN1 + writeback split in token halves, half-outer; half 0's
    chain is emitted at half 1's m=1 and its writeback tiles are
    emitted one-per-close inside half 1's m-loop, so the in-order
    engine queues interleave everything under half 1's matmuls.
"""

import numpy as np

P = 128
D = 1024
N = 4096
NCORES = 8
TOK = N // NCORES  # 512 tokens per core
HT = TOK // 2  # 256-token halves for the y/LN1/writeback pipeline
DK = D // P  # 8 feature tiles
KP = DK // 2  # 4 feature pair-tiles
NJ = N // P  # 32 global token tiles
JP = NJ // 2  # 16 token pair-tiles
EPS = 1e-5
WSCALE = 16.0  # fp8 range scale on B and Wv
ASCALE = 32.0  # fp8 range scale on normalized attnx
SINV = 1.0 / 512.0  # 1/(WSCALE*sqrt(D)) exp logit scale; also 1/(WSCALE*ASCALE)

_cache = {}


def _build_nc(has_cb):
    import concourse.tile as tile
    from concourse import bacc, mybir
    from contextlib import ExitStack

    f32 = mybir.dt.float32
    bf16 = mybir.dt.bfloat16
    f8 = mybir.dt.float8e4
    Exp = mybir.ActivationFunctionType.Exp
    Sqrt = mybir.ActivationFunctionType.Sqrt
    Copy = mybir.ActivationFunctionType.Copy
    Square = mybir.ActivationFunctionType.Square
    DR = mybir.MatmulPerfMode.DoubleRow

    nc = bacc.Bacc("TRN2", target_bir_lowering=False, debug=False, num_devices=NCORES)

    # local shard (T-layout, pre-blocked): bf16 copy carries +bv prefolded
    # (residual only); fp8 copy is pure x for the score path
    xTb = nc.dram_tensor("xTb", [P, DK, TOK], bf16, kind="ExternalInput").ap()
    xT8 = nc.dram_tensor("xT8", [P, KP, 2, TOK], f8, kind="ExternalInput").ap()
    # full x, both layouts, fp8, host pre-permuted into pair-blocked form
    xTg8 = nc.dram_tensor("xTg8", [P, NJ, KP, 2, P], f8, kind="ExternalInput").ap()
    xg8 = nc.dram_tensor("xg8", [P, DK, JP, 2, P], f8, kind="ExternalInput").ap()
    # folded weights (pair-blocked fp8 stationaries)
    B8d = nc.dram_tensor("B8d", [P, DK, KP, 2, P], f8, kind="ExternalInput").ap()
    Wv8 = nc.dram_tensor("Wv8", [P, DK, KP, 2, P], f8, kind="ExternalInput").ap()
    Wg2T = nc.dram_tensor("Wg2T", [P, DK, D], bf16, kind="ExternalInput").ap()
    # [1/g1; 1/g1^2] blocked [P, 2, DK]
    lncon = nc.dram_tensor("lncon", [P, 2, DK], f32, kind="ExternalInput").ap()
    # 1-row-blocked bf16 stationaries for the PE outer-product folds
    s2nr = nc.dram_tensor("s2nr", [1, DK, P], bf16, kind="ExternalInput").ap()
    g1r = nc.dram_tensor("g1r", [1, DK, P], bf16, kind="ExternalInput").ap()
    nb1nr = nc.dram_tensor("nb1nr", [1, DK, P], bf16, kind="ExternalInput").ap()
    cbr = (
        nc.dram_tensor("cbr", [1, DK, P], bf16, kind="ExternalInput").ap()
        if has_cb
        else None
    )
    outT = nc.dram_tensor("outT", [2, P, DK, HT], bf16, kind="ExternalOutput").ap()

    with tile.TileContext(nc) as tc, ExitStack() as ctx:
        ctx.enter_context(
            nc.allow_low_precision("bf16 stat rows; LN-invariant rescale")
        )
        consts = ctx.enter_context(tc.tile_pool(name="consts", bufs=1))
        xin = ctx.enter_context(tc.tile_pool(name="xin", bufs=1))
        bigx = ctx.enter_context(tc.tile_pool(name="bigx", bufs=1))
        wp = ctx.enter_context(tc.tile_pool(name="wp", bufs=1))
        mid = ctx.enter_context(tc.tile_pool(name="mid", bufs=1))
        ev = ctx.enter_context(tc.tile_pool(name="ev", bufs=2))
        ps = ctx.enter_context(tc.tile_pool(name="ps", bufs=3, space="PSUM"))
        pss = ctx.enter_context(tc.tile_pool(name="pss", bufs=3, space="PSUM"))
        psb = ctx.enter_context(tc.tile_pool(name="psb", bufs=2, space="PSUM"))

        # ---- front-critical input DMAs, all on the sync HWDGE ring
        # (in-order): xT8 -> B8d (gates xB) -> xTg8 chunks (gate scores).
        xT8_sb = xin.tile([P, KP, 2, TOK], f8, tag="x8s")
        nc.sync.dma_start(out=xT8_sb, in_=xT8)
        B8_sb = wp.tile([P, DK, KP, 2, P], f8)
        for c in range(4):
            nc.sync.dma_start(
                out=B8_sb[:, 2 * c : 2 * c + 2], in_=B8d[:, 2 * c : 2 * c + 2]
            )
        xTg_sb = bigx.tile([P, NJ, KP, 2, P], f8)
        for c in range(8):
            nc.sync.dma_start(
                out=xTg_sb[:, 4 * c : 4 * c + 4], in_=xTg8[:, 4 * c : 4 * c + 4]
            )

        # ---- constants -------------------------------------------------
        ones8 = consts.tile([P, 2, 16], f8)
        nc.vector.memset(ones8, 1.0)
        ones_b = consts.tile([P, 1], bf16)
        nc.vector.memset(ones_b, 1.0)
        onesr = consts.tile([1, P], bf16)
        nc.vector.memset(onesr, 1.0)
        eps_sb = consts.tile([1, 1], f32)
        nc.vector.memset(eps_sb, EPS)

        from concourse.bass import (
            AP,
            MemorySpace,
            assert_is_scalar,
            assert_partition_dims_match,
        )

        def act_raw(out, in_, func, bias=0.0, scale=1.0):
            eng = nc.scalar
            inputs = [eng.lower_ap(in_)]
            for arg in (bias, scale, 0.0):
                if isinstance(arg, AP):
                    assert_partition_dims_match(arg, in_)
                    assert_is_scalar(arg)
                    assert arg.space == MemorySpace.SBUF
                    inputs.append(eng.lower_ap(arg))
                else:
                    inputs.append(
                        mybir.ImmediateValue(dtype=mybir.dt.float32, value=arg)
                    )
            return eng.add_instruction(
                mybir.InstActivation(
                    name=eng.bass.get_next_instruction_name(),
                    func=func,
                    ins=inputs,
                    outs=[eng.lower_ap(out)],
                )
            )

        Rsqrt = mybir.ActivationFunctionType.Rsqrt
        Recip = mybir.ActivationFunctionType.Reciprocal

        _bc_n = [0]

        def bcast(row_b, tag, width=TOK):
            """[1, w] bf16 -> [P, w] bf16 broadcast via PE outer product."""
            _bc_n[0] += 1
            pt = psb.tile([P, width], f32, tag="bc", name=f"bc_{_bc_n[0]}")
            nc.tensor.matmul(pt, onesr, row_b, start=True, stop=True)
            sb = consts.tile(
                [P, width], bf16, name=f"bcs_{_bc_n[0]}", tag=f"bcs_{tag}"
            )
            nc.vector.tensor_copy(sb, pt)
            return sb

        # ---- xB = (16B) contract x (fp8 DoubleRow) ----------------------
        xB8_sb = mid.tile([P, KP, 2, TOK], f8)
        for m in range(DK):
            pt = ps.tile([P, TOK], f32, tag="pb")
            for k in range(KP):
                nc.tensor.matmul(
                    pt,
                    B8_sb[:, m, k],
                    xT8_sb[:, k],
                    start=(k == 0),
                    stop=(k == KP - 1),
                    perf_mode=DR,
                )
            nc.scalar.activation(xB8_sb[:, m // 2, m % 2, :], pt, Copy)

        # ---- background loads on gpsimd SWDGE, gated behind a tiny copy
        # of xB8 pair 0 so their transfers kick only once the front-
        # critical sync-ring traffic is nearly done. --------------------
        gate_t = ev.tile([P, 2, 1], bf16, tag="gate")
        nc.gpsimd.tensor_copy(gate_t, xB8_sb[:, 0, :, 0:1])
        lncon_sb = consts.tile([P, 2, DK], f32)
        nc.gpsimd.dma_start(out=lncon_sb, in_=lncon)
        s2n_sb = consts.tile([1, DK, P], bf16)
        nc.gpsimd.dma_start(out=s2n_sb, in_=s2nr)
        g1r_sb = consts.tile([1, DK, P], bf16)
        nc.gpsimd.dma_start(out=g1r_sb, in_=g1r)
        nb1n_sb = consts.tile([1, DK, P], bf16)
        nc.gpsimd.dma_start(out=nb1n_sb, in_=nb1nr)
        if has_cb:
            cb_sb = consts.tile([1, DK, P], bf16)
            nc.gpsimd.dma_start(out=cb_sb, in_=cbr)
        xg_sb = bigx.tile([P, DK, JP, 2, P], f8)
        for c in range(8):
            nc.gpsimd.dma_start(out=xg_sb[:, c], in_=xg8[:, c])
        Wv8_sb = wp.tile([P, DK, KP, 2, P], f8)
        nc.gpsimd.dma_start(out=Wv8_sb, in_=Wv8)
        xTb_sb = xin.tile([P, DK, TOK], bf16)
        nc.gpsimd.dma_start(out=xTb_sb, in_=xTb)
        Wg2T_sb = wp.tile([P, DK, D], bf16)
        nc.gpsimd.dma_start(out=Wg2T_sb, in_=Wg2T)
        invg_sb = consts.tile([P, 2, DK], bf16)
        nc.vector.tensor_copy(invg_sb, lncon_sb)

        # ---- scores S^T + exp -> fp8 probs, denominator interleaved ----
        pT8 = mid.tile([P, JP, 2, TOK], f8, tag="big16")
        psd = pss.tile([1, TOK], f32, tag="psm")
        for j in range(NJ):
            pt = ps.tile([P, TOK], f32, tag="pb")
            for k in range(KP):
                nc.tensor.matmul(
                    pt,
                    xTg_sb[:, j, k],
                    xB8_sb[:, k],
                    start=(k == 0),
                    stop=(k == KP - 1),
                    perf_mode=DR,
                )
            nc.scalar.activation(pT8[:, j // 2, j % 2, :], pt, Exp, bias=0.0, scale=SINV)
            if j % 2 == 1:
                nc.tensor.matmul(
                    psd,
                    ones8[:, :, 0:1],
                    pT8[:, j // 2],
                    start=(j == 1),
                    stop=(j == NJ - 1),
                    perf_mode=DR,
                )
        rden32 = consts.tile([1, TOK], bf16)
        act_raw(rden32, psd, Recip, bias=0.0, scale=1.0 / ASCALE)

        # ---- attnx = P @ x (fp8 DoubleRow), normalized to fp8. The rden
        # broadcast matmul is issued AFTER m=0's matmuls so the PE queue
        # doesn't head-of-line block on the scalar reciprocal chain. ----
        attnx8 = xin.tile([P, KP, 2, TOK], f8, tag="x8s", name="attnx8")
        rden_b = None
        for m in range(DK):
            pt = ps.tile([P, TOK], f32, tag="pb")
            for j in range(JP):
                nc.tensor.matmul(
                    pt,
                    xg_sb[:, m, j],
                    pT8[:, j],
                    start=(j == 0),
                    stop=(j == JP - 1),
                    perf_mode=DR,
                )
            if m == 0:
                rden_b = bcast(rden32, "rden")
            nc.vector.tensor_mul(attnx8[:, m // 2, m % 2, :], pt, rden_b)

        # ---- attn_out = attnx @ (16Wv).T / 512 + (x + bv) = res --------
        resb = xin.tile([P, DK, TOK], bf16)
        psm0 = pss.tile([1, TOK], f32, tag="psm")
        psq0 = pss.tile([1, TOK], f32, tag="psm") if has_cb else None
        for m in range(DK):
            pt = ps.tile([P, TOK], f32, tag="pb")
            for k in range(KP):
                nc.tensor.matmul(
                    pt,
                    Wv8_sb[:, m, k],
                    attnx8[:, k],
                    start=(k == 0),
                    stop=(k == KP - 1),
                    perf_mode=DR,
                )
            t1 = ev.tile([P, TOK], f32, tag="sq")
            nc.scalar.activation(t1, pt, Copy, bias=0.0, scale=SINV)
            nc.vector.tensor_add(resb[:, m, :], t1, xTb_sb[:, m, :])
            nc.tensor.matmul(
                psm0, ones_b, resb[:, m, :], start=(m == 0), stop=(m == DK - 1)
            )
            if has_cb:
                sq = ev.tile([P, TOK], bf16, tag="sqb")
                nc.scalar.activation(sq, resb[:, m, :], Square)
                nc.tensor.matmul(
                    psq0, ones_b, sq, start=(m == 0), stop=(m == DK - 1)
                )

        # ---- LN0 scalars for the cfix fold. LN1 is invariant to a
        # per-token scale, so instead of multiplying y by rstd0 we DIVIDE
        # the correction by it: acc2 = y + mu0*s2n (+ std0*cb). --------
        mu0b = consts.tile([1, TOK], bf16, tag="ln_mu0")
        act_raw(mu0b, psm0, Copy, bias=0.0, scale=1.0 / D)
        if has_cb:
            e20 = consts.tile([1, TOK], f32, tag="ln_e2")
            act_raw(e20, psq0, Copy, bias=0.0, scale=1.0 / D)
            mu0f = consts.tile([1, TOK], f32, tag="ln_mu0f")
            act_raw(mu0f, psm0, Copy, bias=0.0, scale=1.0 / D)
            mu20 = consts.tile([1, TOK], f32, tag="ln_mu2")
            nc.scalar.activation(mu20, mu0f, Square)
            nc.vector.tensor_sub(e20, e20, mu20)
            std0b = consts.tile([1, TOK], bf16, tag="ln_std0")
            act_raw(std0b, e20, Sqrt, bias=eps_sb[:])

        # ---- y = res @ Wg2.T (bf16) + cfix fold, token halves ----------
        acc = mid.tile([P, DK, TOK], bf16, tag="big16", name="acc")
        psm1 = [None, None]
        psq1 = [None, None]
        lnrows = [None, None]  # (mu1b, std1b, rstd1_b) per half
        outh_t = [None, None]
        hss = [slice(0, HT), slice(HT, TOK)]

        def emit_y_half(h, hooks=None):
            hs = hss[h]
            psmq = pss.tile([1, TOK], f32, tag="psm", name=f"psmq1{h}")
            psm1[h] = psmq[:, 0:HT]
            psq1[h] = psmq[:, HT:TOK]
            lag = 1 if h == 0 else 0  # let mu0 land before the first
            pend = []  # cfix fold closes a PSUM group
            hooks = hooks or {}
            for m in range(DK):
                pt = ps.tile([P, HT], f32, tag="pb", name=f"y{h}_{m}")
                for k in range(DK):
                    nc.tensor.matmul(
                        pt,
                        Wg2T_sb[:, k, m * P : (m + 1) * P],
                        resb[:, k, hs],
                        start=(k == 0),
                        stop=False,
                    )
                pend.append((m, pt))
                if len(pend) > lag:
                    _close_y(h, hs, *pend.pop(0))
                if m in hooks:
                    hooks[m]()
            while pend:
                _close_y(h, hs, *pend.pop(0))

        def _close_y(h, hs, m, pt):
            nc.tensor.matmul(
                pt, s2n_sb[:, m], mu0b[0:1, hs], start=False,
                stop=not has_cb,
            )
            if has_cb:
                nc.tensor.matmul(
                    pt, cb_sb[:, m], std0b[0:1, hs], start=False, stop=True
                )
            nc.scalar.activation(acc[:, m, hs], pt, Copy)
            sq1 = ev.tile([P, HT], bf16, tag="sqb")
            if m >= DK - 2:  # keep the last squares off the DVE backlog:
                nc.scalar.activation(sq1, acc[:, m, hs], Square)
            else:
                nc.vector.tensor_mul(sq1, acc[:, m, hs], acc[:, m, hs])
            nc.tensor.matmul(
                psm1[h],
                invg_sb[:, 0, m : m + 1],
                acc[:, m, hs],
                start=(m == 0),
                stop=(m == DK - 1),
            )
            nc.tensor.matmul(
                psq1[h],
                invg_sb[:, 1, m : m + 1],
                sq1,
                start=(m == 0),
                stop=(m == DK - 1),
            )

        def emit_chain(h):
            """LN1 scalars for one half: mu1, std1 (feed the PE c3 outer
            products) and rstd1 broadcast."""
            mu1f = consts.tile([1, HT], f32, tag="ln_mu", name=f"mu1{h}")
            act_raw(mu1f, psm1[h], Copy, bias=0.0, scale=1.0 / D)
            e21 = consts.tile([1, HT], f32, tag="ln_e2b", name=f"e21{h}")
            act_raw(e21, psq1[h], Copy, bias=0.0, scale=1.0 / D)
            mu21 = consts.tile([1, HT], f32, tag="ln_mu2b", name=f"mu21{h}")
            nc.scalar.activation(mu21, mu1f, Square)
            nc.vector.tensor_sub(e21, e21, mu21)
            mu1b = consts.tile([1, HT], bf16, tag="ln_mub", name=f"mu1b{h}")
            nc.vector.tensor_copy(mu1b, mu1f)
            std1b = consts.tile([1, HT], bf16, tag="ln_std1", name=f"std1{h}")
            act_raw(std1b, e21, Sqrt, bias=eps_sb[:])
            rstd1 = consts.tile([1, HT], bf16, tag="ln_rstd", name=f"rstd1{h}")
            act_raw(rstd1, e21, Rsqrt, bias=eps_sb[:])
            rstd1_b = bcast(rstd1, "rstd1", width=HT)
            lnrows[h] = (mu1b, std1b, rstd1_b)
            outh_t[h] = mid.tile(
                [P, DK, HT], bf16, tag="outh", bufs=2, name=f"outh{h}"
            )

        def emit_wb_tile(h, m):
            """out[:, m] = rstd1*(acc - c3); c3 = mu1 (x) g1 + std1 (x) (-b1n)."""
            hs = hss[h]
            mu1b, std1b, rstd1_b = lnrows[h]
            c3p = psb.tile([P, HT], f32, tag="bc")
            nc.tensor.matmul(c3p, g1r_sb[:, m], mu1b, start=True, stop=False)
            nc.tensor.matmul(c3p, nb1n_sb[:, m], std1b, start=False, stop=True)
            gp = m in (2, 5)  # gpsimd owns two tiles per half end-to-end
            c2b = ev.tile([P, HT], bf16, tag="ft1", bufs=3)
            if m % 2 == 0 and not gp:
                nc.scalar.activation(c2b, c3p, Copy)
            else:
                nc.vector.tensor_copy(c2b, c3p)
            eng = nc.gpsimd if gp else nc.vector
            t1 = ev.tile([P, HT], bf16, tag="ot", bufs=3)
            eng.tensor_sub(t1, acc[:, m, hs], c2b)
            eng.tensor_mul(outh_t[h][:, m], t1, rstd1_b)
            if m == 3:
                nc.sync.dma_start(out=outT[h, :, 0:4], in_=outh_t[h][:, 0:4])
            elif m == DK - 1:
                nc.sync.dma_start(out=outT[h, :, 4:8], in_=outh_t[h][:, 4:8])

        # half 0 plain; half 1 interleaves half 0's chain (at m=1) and
        # writeback tiles (one per close from m=3) under its matmuls.
        emit_y_half(0)
        wb_state = {"n": 0}

        def _h1_hook_chain():
            emit_chain(0)

        def _h1_hook_wb():
            emit_wb_tile(0, wb_state["n"])
            wb_state["n"] += 1

        emit_y_half(
            1,
            hooks={
                1: _h1_hook_chain,
                3: _h1_hook_wb, 4: _h1_hook_wb, 5: _h1_hook_wb,
                6: _h1_hook_wb, 7: _h1_hook_wb,
            },
        )
        while wb_state["n"] < DK:
            _h1_hook_wb()
        emit_chain(1)
        for m in range(DK):
            emit_wb_tile(1, m)

    nc.finalize()
    return nc


def _get_nc(has_cb):
    key = ("nc", has_cb)
    if key not in _cache:
        _cache[key] = _build_nc(has_cb)
    return _cache[key]


def _pair_block_m(w):
    """[D, M] -> [P, M//P, KP, 2, P] m-major pair-blocked stationary.

    w[d, m] with d = (2*k + i)*P + p, m = mt*P + c lands at
    out[p, mt, k, i, c] so each [2, P] block is contiguous and each
    output-tile's weights are one contiguous DRAM run per partition.
    """
    Dd, M = w.shape
    return np.ascontiguousarray(
        w.reshape(Dd // (2 * P), 2, P, M // P, P).transpose(2, 3, 0, 1, 4)
    )


def _tblock(w):
    """[D, M] -> [P, D//P, M]: d = k*P + p lands at [p, k, :]."""
    Dd, M = w.shape
    return np.ascontiguousarray(w.reshape(Dd // P, P, M).transpose(1, 0, 2))


def _make_in_maps(inputs):
    import ml_dtypes

    bf = ml_dtypes.bfloat16
    f8 = ml_dtypes.float8_e4m3

    x = np.asarray(inputs["x"], dtype=np.float64)
    Wq = np.asarray(inputs["Wq"], np.float64)
    Wk = np.asarray(inputs["Wk"], np.float64)
    Wv = np.asarray(inputs["Wv"], np.float64)
    W1 = np.asarray(inputs["W1"], np.float64)
    W2 = np.asarray(inputs["W2"], np.float64)
    g0 = np.asarray(inputs["g0"], np.float64)
    b0 = np.asarray(inputs["b0"], np.float64)
    b1 = np.asarray(inputs["b1"], np.float64)
    b2 = np.asarray(inputs["b2"], np.float64)

    xf32 = x.astype(np.float32)
    x8 = xf32.astype(f8)
    xT8f = np.ascontiguousarray(xf32.T).astype(f8)

    Wf = W2 @ W1
    Wg = Wf + np.eye(D)
    g1f = np.asarray(inputs["g1"], np.float64)
    Wg2 = Wg * g0[None, :] * g1f[:, None]
    invg = np.stack(
        [
            (1.0 / g1f).astype(np.float32),
            (1.0 / (g1f * g1f)).astype(np.float32),
        ],
        axis=0,
    )  # [2, D]
    s2n = (-Wg2.sum(axis=1)).astype(np.float32)
    cb = ((Wg @ b0 + W2 @ b1 + b2) * g1f).astype(np.float32)
    shared = {
        "B8d": _pair_block_m((WSCALE * (Wq.T @ Wk)).astype(np.float32).astype(f8)),
        "Wv8": _pair_block_m((WSCALE * Wv.T).astype(np.float32).astype(f8)),
        "Wg2T": _tblock(Wg2.T.astype(np.float32).astype(bf)),
        # [P, 2, DK]: row d = m*P + p of each vector at [p, i, m]
        "lncon": np.ascontiguousarray(
            invg.reshape(2, DK, P).transpose(2, 0, 1)
        ),
        "s2nr": np.ascontiguousarray(s2n.reshape(1, DK, P)).astype(bf),
        "g1r": np.ascontiguousarray(
            g1f.astype(np.float32).reshape(1, DK, P)
        ).astype(bf),
        "nb1nr": np.ascontiguousarray(
            (-np.asarray(inputs["b1n"], np.float64))
            .astype(np.float32)
            .reshape(1, DK, P)
        ).astype(bf),
        "cbr": np.ascontiguousarray(cb.reshape(1, DK, P)).astype(bf),
        # scores stationary: [p, jt, k, i, m] = x[jt*P+m, (2k+i)*P+p]
        "xTg8": np.ascontiguousarray(
            xT8f.reshape(KP, 2, P, NJ, P).transpose(2, 3, 0, 1, 4)
        ),
        # attnx stationary: [p, mt, jp, i, m] = x[(2jp+i)*P+p, mt*P+m]
        "xg8": np.ascontiguousarray(
            x8.reshape(JP, 2, P, DK, P).transpose(2, 3, 0, 1, 4)
        ),
    }
    has_cb = bool(np.any(cb != 0.0))
    bvf = np.asarray(inputs["bv"], np.float64)
    xTbv = (x + bvf[None, :]).T.astype(np.float32)
    xT = np.ascontiguousarray(xf32.T)
    in_maps = []
    for c in range(NCORES):
        m = dict(shared)
        m["xTb"] = _tblock(
            np.ascontiguousarray(xTbv[:, c * TOK : (c + 1) * TOK]).astype(bf)
        )
        # moving operand of xB: [p, k, i, t] = x[t, (2k+i)*P+p]
        xTl = np.ascontiguousarray(xT[:, c * TOK : (c + 1) * TOK]).astype(f8)
        m["xT8"] = np.ascontiguousarray(
            xTl.reshape(KP, 2, P, TOK).transpose(2, 0, 1, 3)
        )
        if not has_cb:
            del m["cbr"]
        in_maps.append(m)
    return in_maps, has_cb


def _assemble(res):
    out = np.empty((N, D), dtype=np.float32)
    for c in range(NCORES):
        # outT [2, P, DK, HT] bf16: out[h*HT+t, m*P+p] = arr[h, p, m, t]
        arr = np.asarray(res.results[c]["outT"], dtype=np.float32)
        out[c * TOK : (c + 1) * TOK, :] = arr.transpose(0, 3, 2, 1).reshape(TOK, D)
    return out


def kernel(**inputs):
    from concourse import bass_utils

    in_maps, has_cb = _make_in_maps(inputs)
    nc = _get_nc(has_cb)
    res = bass_utils.run_bass_kernel_spmd(
        nc, in_maps, core_ids=list(range(NCORES)), trace=False
    )
    return _assemble(res)


def run_traced(inputs):
    """Like kernel() but with NTFF tracing; returns (out, exec_time_ns, results)."""
    import hookshim

    hookshim.install()
    from concourse import bass_utils

    in_maps, has_cb = _make_in_maps(inputs)
    nc = _get_nc(has_cb)
    res = bass_utils.run_bass_kernel_spmd(
        nc, in_maps, core_ids=list(range(NCORES)), trace=True
    )
    return _assemble(res), res.exec_time_ns, res


# revision 8
# speedup vs baseline: 1.1712x; 1.0487x over previous
"""Distributed single-head transformer block on 8 TRN2 NeuronCores.

Collective-free restructuring. Algebraic folds done on the host
(weights only):
  - FFN has no activation between its two Linears, so it collapses to a
    single matrix Wf = W2@W1; the residual h folds in as Wg = Wf + I and
    LN0's gamma folds per-column: Wg2 = Wg * g0.
  - Q/K projections collapse into B = Wq.T @ Wk, so scores = x B x.T.
    Each core holds the FULL x (replicated at input-distribution time),
    so there is no K AllGather.
  - attn @ v = (P @ x) @ Wv.T + bv (softmax rows sum to 1), so there is
    no V AllGather either: P @ x uses the same resident full x.
  - LN0 is folded via LN shift/scale invariance: LN1(acc) ==
    LN1(acc/rstd0), so the per-token LN0 correction becomes
    acc2 = y + mu0*s2n (+ std0*cb when biases exist) -- 1-row bf16
    matmuls accumulated INTO the same PSUM group as y. With zero biases
    the whole LN0 variance path vanishes.
  - LN1 writeback uses out = rstd1*(acc - c3), c3 = mu1 (x) g1 +
    std1 (x) (-b1n): c3 rides PE 1-row outer products that need only
    mu1/std1 (ready BEFORE rstd1), so the scalar engine's c2 chain is
    off the tail's critical path.

All large matmuls run in fp8 DoubleRow (2 contraction k-tiles per
instruction, 157 TF/s). The dual-fp8 ldweights ISA check requires each
(2,128) weight pair-block to be contiguous in SBUF, so the host
pre-permutes every stationary operand into [..., 2, 128]-blocked layout;
moving operands are written [..., 2, TOK]-blocked on chip.

Scheduling (v4):
  - sync HWDGE ring (in-order): xT8 -> B8d -> xTg8 in 8 chunks, so xB
    starts on the first kilobytes and scores start on the first chunk.
    The scalar ring carries NO early DMAs (its engine runs the evac/exp
    path). Background tensors ride gpsimd SWDGE, EMITTED after the xB
    phase so the in-order gpsimd queue naturally delays their kicks
    behind a tiny gate-copy of xB8 -- they cannot steal front bandwidth.
  - y + LN1 + writeback split in token halves, half-outer; half 0's
    chain is emitted at half 1's m=1 and its writeback tiles are
    emitted one-per-close inside half 1's m-loop, so the in-order
    engine queues interleave everything under half 1's matmuls.
"""

import numpy as np

P = 128
D = 1024
N = 4096
NCORES = 8
TOK = N // NCORES  # 512 tokens per core
HT = TOK // 2  # 256-token halves for the y/LN1/writeback pipeline
DK = D // P  # 8 feature tiles
KP = DK // 2  # 4 feature pair-tiles
NJ = N // P  # 32 global token tiles
JP = NJ // 2  # 16 token pair-tiles
EPS = 1e-5
WSCALE = 16.0  # fp8 range scale on B and Wv
ASCALE = 32.0  # fp8 range scale on normalized attnx
SINV = 1.0 / 512.0  # 1/(WSCALE*sqrt(D)) exp logit scale; also 1/(WSCALE*ASCALE)

_cache = {}


def _build_nc(has_cb, has_b1n, has_g1):
    import concourse.tile as tile
    from concourse import bacc, mybir
    from contextlib import ExitStack

    f32 = mybir.dt.float32
    bf16 = mybir.dt.bfloat16
    f8 = mybir.dt.float8e4
    Exp = mybir.ActivationFunctionType.Exp
    Sqrt = mybir.ActivationFunctionType.Sqrt
    Copy = mybir.ActivationFunctionType.Copy
    Square = mybir.ActivationFunctionType.Square
    DR = mybir.MatmulPerfMode.DoubleRow

    nc = bacc.Bacc("TRN2", target_bir_lowering=False, debug=False, num_devices=NCORES)

    # local shard (T-layout, pre-blocked): bf16 copy carries +bv prefolded
    # (residual only); fp8 copy is pure x for the score path
    xTb = nc.dram_tensor("xTb", [P, DK, TOK], bf16, kind="ExternalInput").ap()
    xT8 = nc.dram_tensor("xT8", [P, KP, 2, TOK], f8, kind="ExternalInput").ap()
    # full x, both layouts, fp8, host pre-permuted into pair-blocked form
    xTg8 = nc.dram_tensor("xTg8", [P, NJ, KP, 2, P], f8, kind="ExternalInput").ap()
    xg8 = nc.dram_tensor("xg8", [P, DK, JP, 2, P], f8, kind="ExternalInput").ap()
    # folded weights (pair-blocked fp8 stationaries)
    B8d = nc.dram_tensor("B8d", [P, DK, KP, 2, P], f8, kind="ExternalInput").ap()
    Wv8 = nc.dram_tensor("Wv8", [P, DK, KP, 2, P], f8, kind="ExternalInput").ap()
    Wg2T = nc.dram_tensor("Wg2T", [P, DK, D], bf16, kind="ExternalInput").ap()
    # [1/g1; 1/g1^2] blocked [P, 2, DK]
    lncon = nc.dram_tensor("lncon", [P, 2, DK], f32, kind="ExternalInput").ap()
    # 1-row-blocked bf16 stationaries for the PE outer-product folds
    s2nr = nc.dram_tensor("s2nr", [1, DK, P], bf16, kind="ExternalInput").ap()
    g1r = nc.dram_tensor("g1r", [1, DK, P], bf16, kind="ExternalInput").ap()
    nb1nr = nc.dram_tensor("nb1nr", [1, DK, P], bf16, kind="ExternalInput").ap()
    cbr = (
        nc.dram_tensor("cbr", [1, DK, P], bf16, kind="ExternalInput").ap()
        if has_cb
        else None
    )
    outT = nc.dram_tensor("outT", [2, P, DK, HT], bf16, kind="ExternalOutput").ap()

    with tile.TileContext(nc) as tc, ExitStack() as ctx:
        ctx.enter_context(
            nc.allow_low_precision("bf16 stat rows; LN-invariant rescale")
        )
        consts = ctx.enter_context(tc.tile_pool(name="consts", bufs=1))
        xin = ctx.enter_context(tc.tile_pool(name="xin", bufs=1))
        bigx = ctx.enter_context(tc.tile_pool(name="bigx", bufs=1))
        wp = ctx.enter_context(tc.tile_pool(name="wp", bufs=1))
        mid = ctx.enter_context(tc.tile_pool(name="mid", bufs=1))
        ev = ctx.enter_context(tc.tile_pool(name="ev", bufs=2))
        ps = ctx.enter_context(tc.tile_pool(name="ps", bufs=3, space="PSUM"))
        pss = ctx.enter_context(tc.tile_pool(name="pss", bufs=3, space="PSUM"))
        psb = ctx.enter_context(tc.tile_pool(name="psb", bufs=2, space="PSUM"))

        # ---- front-critical input DMAs, all on the sync HWDGE ring
        # (in-order): xT8 -> B8d (gates xB) -> xTg8 chunks (gate scores).
        xT8_sb = xin.tile([P, KP, 2, TOK], f8, tag="x8s")
        nc.sync.dma_start(out=xT8_sb, in_=xT8)
        B8_sb = wp.tile([P, DK, KP, 2, P], f8)
        for c in range(4):
            nc.sync.dma_start(
                out=B8_sb[:, 2 * c : 2 * c + 2], in_=B8d[:, 2 * c : 2 * c + 2]
            )
        xTg_sb = bigx.tile([P, NJ, KP, 2, P], f8)
        for c in range(8):
            nc.sync.dma_start(
                out=xTg_sb[:, 4 * c : 4 * c + 4], in_=xTg8[:, 4 * c : 4 * c + 4]
            )

        # ---- constants -------------------------------------------------
        ones8 = consts.tile([P, 2, 16], f8)
        nc.vector.memset(ones8, 1.0)
        ones_b = consts.tile([P, 1], bf16)
        nc.vector.memset(ones_b, 1.0)
        onesr = consts.tile([1, P], bf16)
        nc.vector.memset(onesr, 1.0)
        eps_sb = consts.tile([1, 1], f32)
        nc.vector.memset(eps_sb, EPS)

        from concourse.bass import (
            AP,
            MemorySpace,
            assert_is_scalar,
            assert_partition_dims_match,
        )

        def act_raw(out, in_, func, bias=0.0, scale=1.0):
            eng = nc.scalar
            inputs = [eng.lower_ap(in_)]
            for arg in (bias, scale, 0.0):
                if isinstance(arg, AP):
                    assert_partition_dims_match(arg, in_)
                    assert_is_scalar(arg)
                    assert arg.space == MemorySpace.SBUF
                    inputs.append(eng.lower_ap(arg))
                else:
                    inputs.append(
                        mybir.ImmediateValue(dtype=mybir.dt.float32, value=arg)
                    )
            return eng.add_instruction(
                mybir.InstActivation(
                    name=eng.bass.get_next_instruction_name(),
                    func=func,
                    ins=inputs,
                    outs=[eng.lower_ap(out)],
                )
            )

        Rsqrt = mybir.ActivationFunctionType.Rsqrt
        Recip = mybir.ActivationFunctionType.Reciprocal

        _bc_n = [0]

        def bcast(row_b, tag, width=TOK):
            """[1, w] bf16 -> [P, w] bf16 broadcast via PE outer product."""
            _bc_n[0] += 1
            pt = psb.tile([P, width], f32, tag="bc", name=f"bc_{_bc_n[0]}")
            nc.tensor.matmul(pt, onesr, row_b, start=True, stop=True)
            sb = consts.tile(
                [P, width], bf16, name=f"bcs_{_bc_n[0]}", tag=f"bcs_{tag}"
            )
            nc.vector.tensor_copy(sb, pt)
            return sb

        # ---- xB = (16B) contract x (fp8 DoubleRow) ----------------------
        xB8_sb = mid.tile([P, KP, 2, TOK], f8)
        for m in range(DK):
            pt = ps.tile([P, TOK], f32, tag="pb")
            for k in range(KP):
                nc.tensor.matmul(
                    pt,
                    B8_sb[:, m, k],
                    xT8_sb[:, k],
                    start=(k == 0),
                    stop=(k == KP - 1),
                    perf_mode=DR,
                )
            nc.scalar.activation(xB8_sb[:, m // 2, m % 2, :], pt, Copy)

        # ---- background loads on gpsimd SWDGE, gated behind a tiny copy
        # of xB8 pair 0 so their transfers kick only once the front-
        # critical sync-ring traffic is nearly done. --------------------
        gate_t = ev.tile([P, 2, 1], bf16, tag="gate")
        nc.gpsimd.tensor_copy(gate_t, xB8_sb[:, 0, :, 0:1])
        lncon_sb = consts.tile([P, 2, DK], f32)
        nc.gpsimd.dma_start(out=lncon_sb, in_=lncon)
        s2n_sb = consts.tile([1, DK, P], bf16)
        nc.gpsimd.dma_start(out=s2n_sb, in_=s2nr)
        g1r_sb = consts.tile([1, DK, P], bf16)
        nc.gpsimd.dma_start(out=g1r_sb, in_=g1r)
        nb1n_sb = consts.tile([1, DK, P], bf16)
        nc.gpsimd.dma_start(out=nb1n_sb, in_=nb1nr)
        if has_cb:
            cb_sb = consts.tile([1, DK, P], bf16)
            nc.gpsimd.dma_start(out=cb_sb, in_=cbr)
        xg_sb = bigx.tile([P, DK, JP, 2, P], f8)
        for c in range(8):
            nc.gpsimd.dma_start(out=xg_sb[:, c], in_=xg8[:, c])
        Wv8_sb = wp.tile([P, DK, KP, 2, P], f8)
        nc.gpsimd.dma_start(out=Wv8_sb, in_=Wv8)
        xTb_sb = xin.tile([P, DK, TOK], bf16)
        nc.gpsimd.dma_start(out=xTb_sb, in_=xTb)
        Wg2T_sb = wp.tile([P, DK, D], bf16)
        nc.gpsimd.dma_start(out=Wg2T_sb, in_=Wg2T)
        invg_sb = consts.tile([P, 2, DK], bf16)
        nc.vector.tensor_copy(invg_sb, lncon_sb)

        # ---- scores S^T + exp -> fp8 probs, denominator interleaved ----
        pT8 = mid.tile([P, JP, 2, TOK], f8, tag="big16")
        psd = pss.tile([1, TOK], f32, tag="psm")
        for j in range(NJ):
            pt = ps.tile([P, TOK], f32, tag="pb")
            for k in range(KP):
                nc.tensor.matmul(
                    pt,
                    xTg_sb[:, j, k],
                    xB8_sb[:, k],
                    start=(k == 0),
                    stop=(k == KP - 1),
                    perf_mode=DR,
                )
            nc.scalar.activation(pT8[:, j // 2, j % 2, :], pt, Exp, bias=0.0, scale=SINV)
            if j % 2 == 1:
                nc.tensor.matmul(
                    psd,
                    ones8[:, :, 0:1],
                    pT8[:, j // 2],
                    start=(j == 1),
                    stop=(j == NJ - 1),
                    perf_mode=DR,
                )
        rden32 = consts.tile([1, TOK], bf16)
        act_raw(rden32, psd, Recip, bias=0.0, scale=1.0 / ASCALE)

        # ---- attnx = P @ x (fp8 DoubleRow), normalized to fp8. The rden
        # broadcast matmul is issued AFTER m=0's matmuls so the PE queue
        # doesn't head-of-line block on the scalar reciprocal chain. ----
        attnx8 = xin.tile([P, KP, 2, TOK], f8, tag="x8s", name="attnx8")
        rden_b = None
        for m in range(DK):
            pt = ps.tile([P, TOK], f32, tag="pb")
            for j in range(JP):
                nc.tensor.matmul(
                    pt,
                    xg_sb[:, m, j],
                    pT8[:, j],
                    start=(j == 0),
                    stop=(j == JP - 1),
                    perf_mode=DR,
                )
            if m == 0:
                rden_b = bcast(rden32, "rden")
            nc.vector.tensor_mul(attnx8[:, m // 2, m % 2, :], pt, rden_b)

        # ---- attn_out = attnx @ (16Wv).T / 512 + (x + bv) = res --------
        resb = xin.tile([P, DK, TOK], bf16)
        psm0 = pss.tile([1, TOK], f32, tag="psm")
        psq0 = pss.tile([1, TOK], f32, tag="psm") if has_cb else None
        for m in range(DK):
            pt = ps.tile([P, TOK], f32, tag="pb")
            for k in range(KP):
                nc.tensor.matmul(
                    pt,
                    Wv8_sb[:, m, k],
                    attnx8[:, k],
                    start=(k == 0),
                    stop=(k == KP - 1),
                    perf_mode=DR,
                )
            t1 = ev.tile([P, TOK], f32, tag="sq")
            nc.scalar.activation(t1, pt, Copy, bias=0.0, scale=SINV)
            nc.vector.tensor_add(resb[:, m, :], t1, xTb_sb[:, m, :])
            nc.tensor.matmul(
                psm0, ones_b, resb[:, m, :], start=(m == 0), stop=(m == DK - 1)
            )
            if has_cb:
                sq = ev.tile([P, TOK], bf16, tag="sqb")
                nc.scalar.activation(sq, resb[:, m, :], Square)
                nc.tensor.matmul(
                    psq0, ones_b, sq, start=(m == 0), stop=(m == DK - 1)
                )

        # ---- LN0 scalars for the cfix fold. LN1 is invariant to a
        # per-token scale, so instead of multiplying y by rstd0 we DIVIDE
        # the correction by it: acc2 = y + mu0*s2n (+ std0*cb). --------
        mu0b = consts.tile([1, TOK], bf16, tag="ln_mu0")
        act_raw(mu0b, psm0, Copy, bias=0.0, scale=1.0 / D)
        if has_cb:
            e20 = consts.tile([1, TOK], f32, tag="ln_e2")
            act_raw(e20, psq0, Copy, bias=0.0, scale=1.0 / D)
            mu0f = consts.tile([1, TOK], f32, tag="ln_mu0f")
            act_raw(mu0f, psm0, Copy, bias=0.0, scale=1.0 / D)
            mu20 = consts.tile([1, TOK], f32, tag="ln_mu2")
            nc.scalar.activation(mu20, mu0f, Square)
            nc.vector.tensor_sub(e20, e20, mu20)
            std0b = consts.tile([1, TOK], bf16, tag="ln_std0")
            act_raw(std0b, e20, Sqrt, bias=eps_sb[:])

        # ---- y = res @ Wg2.T (bf16) + cfix fold, token halves ----------
        acc = mid.tile([P, DK, TOK], bf16, tag="big16", name="acc")
        psm1 = [None, None]
        psq1 = [None, None]
        lnrows = [None, None]  # (mu1b, std1b, rstd1_b) per half
        outh_t = [None, None]
        hss = [slice(0, HT), slice(HT, TOK)]

        def emit_y_half(h, hooks=None):
            hs = hss[h]
            psmq = pss.tile([1, TOK], f32, tag="psm", name=f"psmq1{h}")
            psm1[h] = psmq[:, 0:HT]
            psq1[h] = psmq[:, HT:TOK]
            lag = 1 if h == 0 else 0  # let mu0 land before the first
            pend = []  # cfix fold closes a PSUM group
            hooks = hooks or {}
            for m in range(DK):
                pt = ps.tile([P, HT], f32, tag="pb", name=f"y{h}_{m}")
                for k in range(DK):
                    nc.tensor.matmul(
                        pt,
                        Wg2T_sb[:, k, m * P : (m + 1) * P],
                        resb[:, k, hs],
                        start=(k == 0),
                        stop=False,
                    )
                pend.append((m, pt))
                if len(pend) > lag:
                    _close_y(h, hs, *pend.pop(0))
                if m in hooks:
                    hooks[m]()
            while pend:
                _close_y(h, hs, *pend.pop(0))

        def _close_y(h, hs, m, pt):
            nc.tensor.matmul(
                pt, s2n_sb[:, m], mu0b[0:1, hs], start=False,
                stop=not has_cb,
            )
            if has_cb:
                nc.tensor.matmul(
                    pt, cb_sb[:, m], std0b[0:1, hs], start=False, stop=True
                )
            nc.scalar.activation(acc[:, m, hs], pt, Copy)
            sq1 = ev.tile([P, HT], bf16, tag="sqb")
            if m >= DK - 2:  # keep the last squares off the DVE backlog:
                nc.scalar.activation(sq1, acc[:, m, hs], Square)
            else:
                nc.vector.tensor_mul(sq1, acc[:, m, hs], acc[:, m, hs])
            nc.tensor.matmul(
                psm1[h],
                invg_sb[:, 0, m : m + 1],
                acc[:, m, hs],
                start=(m == 0),
                stop=(m == DK - 1),
            )
            nc.tensor.matmul(
                psq1[h],
                invg_sb[:, 1, m : m + 1],
                sq1,
                start=(m == 0),
                stop=(m == DK - 1),
            )

        def emit_chain(h):
            """LN1 scalars for one half: mu1 (and std1 when b1n exists,
            via (e21+eps)*rstd1 -- no Sqrt table load) + rstd1 bcast."""
            mu1f = consts.tile([1, HT], f32, tag="ln_mu", name=f"mu1{h}")
            act_raw(mu1f, psm1[h], Copy, bias=0.0, scale=1.0 / D)
            e21 = consts.tile([1, HT], f32, tag="ln_e2b", name=f"e21{h}")
            act_raw(e21, psq1[h], Copy, bias=0.0, scale=1.0 / D)
            mu21 = consts.tile([1, HT], f32, tag="ln_mu2b", name=f"mu21{h}")
            nc.scalar.activation(mu21, mu1f, Square)
            nc.vector.tensor_sub(e21, e21, mu21)
            rstd1 = consts.tile([1, HT], bf16, tag="ln_rstd", name=f"rstd1{h}")
            act_raw(rstd1, e21, Rsqrt, bias=eps_sb[:])
            rstd1_b = bcast(rstd1, "rstd1", width=HT)
            mu1b = consts.tile([1, HT], bf16, tag="ln_mub", name=f"mu1b{h}")
            nc.vector.tensor_copy(mu1b, mu1f)
            std1b = None
            if has_b1n:
                std1b = consts.tile([1, HT], bf16, tag="ln_std1", name=f"std1{h}")
                nc.vector.scalar_tensor_tensor(
                    std1b, e21, EPS, rstd1,
                    op0=mybir.AluOpType.add, op1=mybir.AluOpType.mult,
                )
            mu1_b = None
            if not has_g1 and not has_b1n:
                # identity LN params: c3 == mu1 broadcast, shared by all m
                mu1_b = bcast(mu1b, "mu1", width=HT)
            lnrows[h] = (mu1b, std1b, rstd1_b, mu1_b)
            outh_t[h] = mid.tile(
                [P, DK, HT], bf16, tag="outh", bufs=2, name=f"outh{h}"
            )

        def emit_wb_tile(h, m):
            """out[:, m] = rstd1*(acc - c3); c3 = mu1 (x) g1 + std1 (x) (-b1n).
            With identity LN params c3 is just the shared mu1 broadcast."""
            hs = hss[h]
            mu1b, std1b, rstd1_b, mu1_b = lnrows[h]
            if mu1_b is not None:
                c2b = mu1_b
            else:
                c3p = psb.tile([P, HT], f32, tag="bc")
                nc.tensor.matmul(
                    c3p, g1r_sb[:, m], mu1b, start=True, stop=not has_b1n
                )
                if has_b1n:
                    nc.tensor.matmul(
                        c3p, nb1n_sb[:, m], std1b, start=False, stop=True
                    )
                c2b = ev.tile([P, HT], bf16, tag="ft1", bufs=3)
                if m % 2 == 0:
                    nc.scalar.activation(c2b, c3p, Copy)
                else:
                    nc.vector.tensor_copy(c2b, c3p)
            gp = m in (2, 5)  # gpsimd owns two tiles per half end-to-end
            eng = nc.gpsimd if gp else nc.vector
            t1 = ev.tile([P, HT], bf16, tag="ot", bufs=3)
            eng.tensor_sub(t1, acc[:, m, hs], c2b)
            eng.tensor_mul(outh_t[h][:, m], t1, rstd1_b)
            if m % 2 == 1:  # flush every 2 tiles so the last DMA is small
                nc.sync.dma_start(
                    out=outT[h, :, m - 1 : m + 1],
                    in_=outh_t[h][:, m - 1 : m + 1],
                )

        # half 0 plain; half 1 interleaves half 0's chain (at m=1) and
        # writeback tiles (one per close from m=3) under its matmuls.
        emit_y_half(0)
        wb_state = {"n": 0}

        def _h1_hook_chain():
            emit_chain(0)

        def _h1_hook_wb():
            emit_wb_tile(0, wb_state["n"])
            wb_state["n"] += 1

        emit_y_half(
            1,
            hooks={
                1: _h1_hook_chain,
                3: _h1_hook_wb, 4: _h1_hook_wb, 5: _h1_hook_wb,
                6: _h1_hook_wb, 7: _h1_hook_wb,
            },
        )
        while wb_state["n"] < DK:
            _h1_hook_wb()
        emit_chain(1)
        for m in range(DK):
            emit_wb_tile(1, m)

    nc.finalize()
    return nc


def _get_nc(flags):
    key = ("nc",) + flags
    if key not in _cache:
        _cache[key] = _build_nc(*flags)
    return _cache[key]


def _pair_block_m(w):
    """[D, M] -> [P, M//P, KP, 2, P] m-major pair-blocked stationary.

    w[d, m] with d = (2*k + i)*P + p, m = mt*P + c lands at
    out[p, mt, k, i, c] so each [2, P] block is contiguous and each
    output-tile's weights are one contiguous DRAM run per partition.
    """
    Dd, M = w.shape
    return np.ascontiguousarray(
        w.reshape(Dd // (2 * P), 2, P, M // P, P).transpose(2, 3, 0, 1, 4)
    )


def _tblock(w):
    """[D, M] -> [P, D//P, M]: d = k*P + p lands at [p, k, :]."""
    Dd, M = w.shape
    return np.ascontiguousarray(w.reshape(Dd // P, P, M).transpose(1, 0, 2))


def _make_in_maps(inputs):
    import ml_dtypes

    bf = ml_dtypes.bfloat16
    f8 = ml_dtypes.float8_e4m3

    x = np.asarray(inputs["x"], dtype=np.float64)
    Wq = np.asarray(inputs["Wq"], np.float64)
    Wk = np.asarray(inputs["Wk"], np.float64)
    Wv = np.asarray(inputs["Wv"], np.float64)
    W1 = np.asarray(inputs["W1"], np.float64)
    W2 = np.asarray(inputs["W2"], np.float64)
    g0 = np.asarray(inputs["g0"], np.float64)
    b0 = np.asarray(inputs["b0"], np.float64)
    b1 = np.asarray(inputs["b1"], np.float64)
    b2 = np.asarray(inputs["b2"], np.float64)

    xf32 = x.astype(np.float32)
    x8 = xf32.astype(f8)
    xT8f = np.ascontiguousarray(xf32.T).astype(f8)

    Wf = W2 @ W1
    Wg = Wf + np.eye(D)
    g1f = np.asarray(inputs["g1"], np.float64)
    Wg2 = Wg * g0[None, :] * g1f[:, None]
    invg = np.stack(
        [
            (1.0 / g1f).astype(np.float32),
            (1.0 / (g1f * g1f)).astype(np.float32),
        ],
        axis=0,
    )  # [2, D]
    s2n = (-Wg2.sum(axis=1)).astype(np.float32)
    cb = ((Wg @ b0 + W2 @ b1 + b2) * g1f).astype(np.float32)
    shared = {
        "B8d": _pair_block_m((WSCALE * (Wq.T @ Wk)).astype(np.float32).astype(f8)),
        "Wv8": _pair_block_m((WSCALE * Wv.T).astype(np.float32).astype(f8)),
        "Wg2T": _tblock(Wg2.T.astype(np.float32).astype(bf)),
        # [P, 2, DK]: row d = m*P + p of each vector at [p, i, m]
        "lncon": np.ascontiguousarray(
            invg.reshape(2, DK, P).transpose(2, 0, 1)
        ),
        "s2nr": np.ascontiguousarray(s2n.reshape(1, DK, P)).astype(bf),
        "g1r": np.ascontiguousarray(
            g1f.astype(np.float32).reshape(1, DK, P)
        ).astype(bf),
        "nb1nr": np.ascontiguousarray(
            (-np.asarray(inputs["b1n"], np.float64))
            .astype(np.float32)
            .reshape(1, DK, P)
        ).astype(bf),
        "cbr": np.ascontiguousarray(cb.reshape(1, DK, P)).astype(bf),
        # scores stationary: [p, jt, k, i, m] = x[jt*P+m, (2k+i)*P+p]
        "xTg8": np.ascontiguousarray(
            xT8f.reshape(KP, 2, P, NJ, P).transpose(2, 3, 0, 1, 4)
        ),
        # attnx stationary: [p, mt, jp, i, m] = x[(2jp+i)*P+p, mt*P+m]
        "xg8": np.ascontiguousarray(
            x8.reshape(JP, 2, P, DK, P).transpose(2, 3, 0, 1, 4)
        ),
    }
    has_cb = bool(np.any(cb != 0.0))
    bvf = np.asarray(inputs["bv"], np.float64)
    xTbv = (x + bvf[None, :]).T.astype(np.float32)
    xT = np.ascontiguousarray(xf32.T)
    in_maps = []
    for c in range(NCORES):
        m = dict(shared)
        m["xTb"] = _tblock(
            np.ascontiguousarray(xTbv[:, c * TOK : (c + 1) * TOK]).astype(bf)
        )
        # moving operand of xB: [p, k, i, t] = x[t, (2k+i)*P+p]
        xTl = np.ascontiguousarray(xT[:, c * TOK : (c + 1) * TOK]).astype(f8)
        m["xT8"] = np.ascontiguousarray(
            xTl.reshape(KP, 2, P, TOK).transpose(2, 0, 1, 3)
        )
        if not has_cb:
            del m["cbr"]
        in_maps.append(m)
    flags = (
        has_cb,
        bool(np.any(np.asarray(inputs["b1n"]) != 0.0)),
        bool(np.any(g1f != 1.0)),
    )
    return in_maps, flags


def _assemble(res):
    out = np.empty((N, D), dtype=np.float32)
    for c in range(NCORES):
        # outT [2, P, DK, HT] bf16: out[h*HT+t, m*P+p] = arr[h, p, m, t]
        arr = np.asarray(res.results[c]["outT"], dtype=np.float32)
        out[c * TOK : (c + 1) * TOK, :] = arr.transpose(0, 3, 2, 1).reshape(TOK, D)
    return out


def kernel(**inputs):
    from concourse import bass_utils

    in_maps, flags = _make_in_maps(inputs)
    nc = _get_nc(flags)
    res = bass_utils.run_bass_kernel_spmd(
        nc, in_maps, core_ids=list(range(NCORES)), trace=False
    )
    return _assemble(res)


def run_traced(inputs):
    """Like kernel() but with NTFF tracing; returns (out, exec_time_ns, results)."""
    import hookshim

    hookshim.install()
    from concourse import bass_utils

    in_maps, flags = _make_in_maps(inputs)
    nc = _get_nc(flags)
    res = bass_utils.run_bass_kernel_spmd(
        nc, in_maps, core_ids=list(range(NCORES)), trace=True
    )
    return _assemble(res), res.exec_time_ns, res


# revision 9
# speedup vs baseline: 1.1813x; 1.0086x over previous
"""Distributed single-head transformer block on 8 TRN2 NeuronCores.

Collective-free restructuring. Algebraic folds done on the host
(weights only):
  - FFN has no activation between its two Linears, so it collapses to a
    single matrix Wf = W2@W1; the residual h folds in as Wg = Wf + I and
    LN0's gamma folds per-column: Wg2 = Wg * g0.
  - Q/K projections collapse into B = Wq.T @ Wk, so scores = x B x.T.
    Each core holds the FULL x (replicated at input-distribution time),
    so there is no K AllGather.
  - attn @ v = (P @ x) @ Wv.T + bv (softmax rows sum to 1), so there is
    no V AllGather either: P @ x uses the same resident full x.
  - LN0 folds via LN scale invariance: LN1(acc) == LN1(acc/rstd0), so
    the LN0 correction becomes acc2 = y + mu0*s2n (+ std0*cb with
    nonzero biases) -- 1-row bf16 matmuls accumulated INTO the y PSUM.
  - LN1's MEAN also folds into the y PSUM: mu1 = (wfold @ res)/D with
    wfold = Wg2.T(1/g1) + (sum(s2n/g1)/D)*ones is just another weight
    row, accumulated during the Wv phase. Subtracting g1 (x) mu1 as a
    fold matmul leaves acc CENTERED, so the LN1 chain is a single
    Rsqrt straight off the variance PSUM and the writeback is one
    multiply per tile (+ b1n scalar-add when present).

All large matmuls run in fp8 DoubleRow (2 contraction k-tiles per
instruction, 157 TF/s). The dual-fp8 ldweights ISA check requires each
(2,128) weight pair-block to be contiguous in SBUF, so the host
pre-permutes every stationary operand into [..., 2, 128]-blocked layout;
moving operands are written [..., 2, TOK]-blocked on chip.

Scheduling (v6):
  - sync HWDGE ring (in-order): xT8 -> B8d -> xTg8 in 8 chunks; scalar
    ring carries no early DMAs; background tensors ride gpsimd SWDGE
    emitted after the xB phase (gated by a copy of xB8) so they cannot
    steal front bandwidth.
  - y + LN1 + writeback split in token halves, half-outer; half 0's
    epilogue is emitted inside half 1's m-loop so the in-order engine
    queues interleave it under half 1's matmuls.
"""

import numpy as np

P = 128
D = 1024
N = 4096
NCORES = 8
TOK = N // NCORES  # 512 tokens per core
HT = TOK // 2  # 256-token halves for the y/LN1/writeback pipeline
DK = D // P  # 8 feature tiles
KP = DK // 2  # 4 feature pair-tiles
NJ = N // P  # 32 global token tiles
JP = NJ // 2  # 16 token pair-tiles
EPS = 1e-5
WSCALE = 16.0  # fp8 range scale on B and Wv
ASCALE = 32.0  # fp8 range scale on normalized attnx
SINV = 1.0 / 512.0  # 1/(WSCALE*sqrt(D)) exp logit scale; also 1/(WSCALE*ASCALE)

_cache = {}


def _build_nc(has_cb, has_b1n):
    import concourse.tile as tile
    from concourse import bacc, mybir
    from contextlib import ExitStack

    f32 = mybir.dt.float32
    bf16 = mybir.dt.bfloat16
    f8 = mybir.dt.float8e4
    Exp = mybir.ActivationFunctionType.Exp
    Sqrt = mybir.ActivationFunctionType.Sqrt
    Copy = mybir.ActivationFunctionType.Copy
    Square = mybir.ActivationFunctionType.Square
    DR = mybir.MatmulPerfMode.DoubleRow

    nc = bacc.Bacc("TRN2", target_bir_lowering=False, debug=False, num_devices=NCORES)

    # local shard (T-layout, pre-blocked): bf16 copy carries +bv prefolded
    # (residual only); fp8 copy is pure x for the score path
    xTb = nc.dram_tensor("xTb", [P, DK, TOK], bf16, kind="ExternalInput").ap()
    xT8 = nc.dram_tensor("xT8", [P, KP, 2, TOK], f8, kind="ExternalInput").ap()
    # full x, both layouts, fp8, host pre-permuted into pair-blocked form
    xTg8 = nc.dram_tensor("xTg8", [P, NJ, KP, 2, P], f8, kind="ExternalInput").ap()
    xg8 = nc.dram_tensor("xg8", [P, DK, JP, 2, P], f8, kind="ExternalInput").ap()
    # folded weights (pair-blocked fp8 stationaries)
    B8d = nc.dram_tensor("B8d", [P, DK, KP, 2, P], f8, kind="ExternalInput").ap()
    Wv8 = nc.dram_tensor("Wv8", [P, DK, KP, 2, P], f8, kind="ExternalInput").ap()
    Wg2T = nc.dram_tensor("Wg2T", [P, DK, D], bf16, kind="ExternalInput").ap()
    # [wfold; invg2; b1n] blocked [P, 3, DK] (per-partition columns)
    lncon = nc.dram_tensor("lncon", [P, 3, DK], f32, kind="ExternalInput").ap()
    # 1-row-blocked bf16 stationaries for the PE outer-product folds
    s2nr = nc.dram_tensor("s2nr", [1, DK, P], bf16, kind="ExternalInput").ap()
    g1r = nc.dram_tensor("g1r", [1, DK, P], bf16, kind="ExternalInput").ap()
    cbr = (
        nc.dram_tensor("cbr", [1, DK, P], bf16, kind="ExternalInput").ap()
        if has_cb
        else None
    )
    outT = nc.dram_tensor("outT", [2, P, DK, HT], bf16, kind="ExternalOutput").ap()

    with tile.TileContext(nc) as tc, ExitStack() as ctx:
        ctx.enter_context(
            nc.allow_low_precision("bf16 stat rows; LN-invariant rescale")
        )
        consts = ctx.enter_context(tc.tile_pool(name="consts", bufs=1))
        xin = ctx.enter_context(tc.tile_pool(name="xin", bufs=1))
        bigx = ctx.enter_context(tc.tile_pool(name="bigx", bufs=1))
        wp = ctx.enter_context(tc.tile_pool(name="wp", bufs=1))
        mid = ctx.enter_context(tc.tile_pool(name="mid", bufs=1))
        ev = ctx.enter_context(tc.tile_pool(name="ev", bufs=2))
        ps = ctx.enter_context(tc.tile_pool(name="ps", bufs=3, space="PSUM"))
        pss = ctx.enter_context(tc.tile_pool(name="pss", bufs=3, space="PSUM"))
        psb = ctx.enter_context(tc.tile_pool(name="psb", bufs=2, space="PSUM"))

        # ---- front-critical input DMAs, all on the sync HWDGE ring
        # (in-order): xT8 -> B8d (gates xB) -> xTg8 chunks (gate scores).
        xT8_sb = xin.tile([P, KP, 2, TOK], f8, tag="x8s")
        nc.sync.dma_start(out=xT8_sb, in_=xT8)
        B8_sb = wp.tile([P, DK, KP, 2, P], f8)
        for c in range(4):
            nc.sync.dma_start(
                out=B8_sb[:, 2 * c : 2 * c + 2], in_=B8d[:, 2 * c : 2 * c + 2]
            )
        xTg_sb = bigx.tile([P, NJ, KP, 2, P], f8)
        for c in range(8):
            nc.sync.dma_start(
                out=xTg_sb[:, 4 * c : 4 * c + 4], in_=xTg8[:, 4 * c : 4 * c + 4]
            )

        # ---- constants -------------------------------------------------
        ones8 = consts.tile([P, 2, 16], f8)
        nc.vector.memset(ones8, 1.0)
        ones_b = consts.tile([P, 1], bf16)
        nc.vector.memset(ones_b, 1.0)
        onesr = consts.tile([1, P], bf16)
        nc.vector.memset(onesr, 1.0)
        eps_sb = consts.tile([1, 1], f32)
        nc.vector.memset(eps_sb, EPS)

        from concourse.bass import (
            AP,
            MemorySpace,
            assert_is_scalar,
            assert_partition_dims_match,
        )

        def act_raw(out, in_, func, bias=0.0, scale=1.0):
            eng = nc.scalar
            inputs = [eng.lower_ap(in_)]
            for arg in (bias, scale, 0.0):
                if isinstance(arg, AP):
                    assert_partition_dims_match(arg, in_)
                    assert_is_scalar(arg)
                    assert arg.space == MemorySpace.SBUF
                    inputs.append(eng.lower_ap(arg))
                else:
                    inputs.append(
                        mybir.ImmediateValue(dtype=mybir.dt.float32, value=arg)
                    )
            return eng.add_instruction(
                mybir.InstActivation(
                    name=eng.bass.get_next_instruction_name(),
                    func=func,
                    ins=inputs,
                    outs=[eng.lower_ap(out)],
                )
            )

        Rsqrt = mybir.ActivationFunctionType.Rsqrt
        Recip = mybir.ActivationFunctionType.Reciprocal

        _bc_n = [0]

        def bcast(row_b, tag, width=TOK):
            """[1, w] bf16 -> [P, w] bf16 broadcast via PE outer product."""
            _bc_n[0] += 1
            pt = psb.tile([P, width], f32, tag="bc", name=f"bc_{_bc_n[0]}")
            nc.tensor.matmul(pt, onesr, row_b, start=True, stop=True)
            sb = consts.tile(
                [P, width], bf16, name=f"bcs_{_bc_n[0]}", tag=f"bcs_{tag}"
            )
            nc.vector.tensor_copy(sb, pt)
            return sb

        # ---- xB = (16B) contract x (fp8 DoubleRow) ----------------------
        xB8_sb = mid.tile([P, KP, 2, TOK], f8)
        for m in range(DK):
            pt = ps.tile([P, TOK], f32, tag="pb")
            for k in range(KP):
                nc.tensor.matmul(
                    pt,
                    B8_sb[:, m, k],
                    xT8_sb[:, k],
                    start=(k == 0),
                    stop=(k == KP - 1),
                    perf_mode=DR,
                )
            nc.scalar.activation(xB8_sb[:, m // 2, m % 2, :], pt, Copy)

        # ---- background loads on gpsimd SWDGE, gated behind a tiny copy
        # of xB8 pair 0 so their transfers kick only once the front-
        # critical sync-ring traffic is nearly done. --------------------
        gate_t = ev.tile([P, 2, 1], bf16, tag="gate")
        nc.gpsimd.tensor_copy(gate_t, xB8_sb[:, 0, :, 0:1])
        lncon_sb = consts.tile([P, 3, DK], f32)
        nc.gpsimd.dma_start(out=lncon_sb, in_=lncon)
        s2n_sb = consts.tile([1, DK, P], bf16)
        nc.gpsimd.dma_start(out=s2n_sb, in_=s2nr)
        g1r_sb = consts.tile([1, DK, P], bf16)
        nc.gpsimd.dma_start(out=g1r_sb, in_=g1r)
        if has_cb:
            cb_sb = consts.tile([1, DK, P], bf16)
            nc.gpsimd.dma_start(out=cb_sb, in_=cbr)
        xg_sb = bigx.tile([P, DK, JP, 2, P], f8)
        for c in range(8):
            nc.gpsimd.dma_start(out=xg_sb[:, c], in_=xg8[:, c])
        Wv8_sb = wp.tile([P, DK, KP, 2, P], f8)
        nc.gpsimd.dma_start(out=Wv8_sb, in_=Wv8)
        xTb_sb = xin.tile([P, DK, TOK], bf16)
        nc.gpsimd.dma_start(out=xTb_sb, in_=xTb)
        Wg2T_sb = wp.tile([P, DK, D], bf16)
        nc.gpsimd.dma_start(out=Wg2T_sb, in_=Wg2T)
        # wfold/invg2 bf16 per-partition stationaries; b1n f32 scalars
        wfold_sb = consts.tile([P, 1, DK], bf16)
        nc.vector.tensor_copy(wfold_sb, lncon_sb[:, 0:1])
        invg2_sb = consts.tile([P, 1, DK], bf16)
        nc.vector.tensor_copy(invg2_sb, lncon_sb[:, 1:2])
        b1n_sb = lncon_sb[:, 2]

        # ---- scores S^T + exp -> fp8 probs, denominator interleaved ----
        pT8 = mid.tile([P, JP, 2, TOK], f8, tag="big16")
        psd = pss.tile([1, TOK], f32, tag="psm")
        for j in range(NJ):
            pt = ps.tile([P, TOK], f32, tag="pb")
            for k in range(KP):
                nc.tensor.matmul(
                    pt,
                    xTg_sb[:, j, k],
                    xB8_sb[:, k],
                    start=(k == 0),
                    stop=(k == KP - 1),
                    perf_mode=DR,
                )
            nc.scalar.activation(pT8[:, j // 2, j % 2, :], pt, Exp, bias=0.0, scale=SINV)
            if j % 2 == 1:
                nc.tensor.matmul(
                    psd,
                    ones8[:, :, 0:1],
                    pT8[:, j // 2],
                    start=(j == 1),
                    stop=(j == NJ - 1),
                    perf_mode=DR,
                )
        rden32 = consts.tile([1, TOK], bf16)
        act_raw(rden32, psd, Recip, bias=0.0, scale=1.0 / ASCALE)

        # ---- attnx = P @ x (fp8 DoubleRow), normalized to fp8. The rden
        # broadcast matmul is issued AFTER m=0's matmuls so the PE queue
        # doesn't head-of-line block on the scalar reciprocal chain. ----
        attnx8 = xin.tile([P, KP, 2, TOK], f8, tag="x8s", name="attnx8")
        rden_b = None
        for m in range(DK):
            pt = ps.tile([P, TOK], f32, tag="pb")
            for j in range(JP):
                nc.tensor.matmul(
                    pt,
                    xg_sb[:, m, j],
                    pT8[:, j],
                    start=(j == 0),
                    stop=(j == JP - 1),
                    perf_mode=DR,
                )
            if m == 0:
                rden_b = bcast(rden32, "rden")
            nc.vector.tensor_mul(attnx8[:, m // 2, m % 2, :], pt, rden_b)

        # ---- attn_out = attnx @ (16Wv).T / 512 + (x + bv) = res.
        # psm0 (ones row) and psmW (wfold row) accumulate here: they
        # feed mu0 and the LN1 mean fold. --------------------------------
        resb = xin.tile([P, DK, TOK], bf16)
        psm0 = pss.tile([1, TOK], f32, tag="psm")
        psmW = pss.tile([1, TOK], f32, tag="psm")
        psq0 = pss.tile([1, TOK], f32, tag="psm") if has_cb else None
        for m in range(DK):
            pt = ps.tile([P, TOK], f32, tag="pb")
            for k in range(KP):
                nc.tensor.matmul(
                    pt,
                    Wv8_sb[:, m, k],
                    attnx8[:, k],
                    start=(k == 0),
                    stop=(k == KP - 1),
                    perf_mode=DR,
                )
            t1 = ev.tile([P, TOK], f32, tag="sq")
            nc.scalar.activation(t1, pt, Copy, bias=0.0, scale=SINV)
            nc.vector.tensor_add(resb[:, m, :], t1, xTb_sb[:, m, :])
            nc.tensor.matmul(
                psm0, ones_b, resb[:, m, :], start=(m == 0), stop=(m == DK - 1)
            )
            nc.tensor.matmul(
                psmW,
                wfold_sb[:, 0, m : m + 1],
                resb[:, m, :],
                start=(m == 0),
                stop=(m == DK - 1),
            )
            if has_cb:
                sq = ev.tile([P, TOK], bf16, tag="sqb")
                nc.scalar.activation(sq, resb[:, m, :], Square)
                nc.tensor.matmul(
                    psq0, ones_b, sq, start=(m == 0), stop=(m == DK - 1)
                )

        # ---- LN0 / LN1-mean scalars feeding the y-PSUM folds -----------
        mu0b = consts.tile([1, TOK], bf16, tag="ln_mu0")
        act_raw(mu0b, psm0, Copy, bias=0.0, scale=1.0 / D)
        nmu1 = consts.tile([1, TOK], bf16, tag="ln_nmu1")
        act_raw(nmu1, psmW, Copy, bias=0.0, scale=-1.0 / D)
        if has_cb:
            e20 = consts.tile([1, TOK], f32, tag="ln_e2")
            act_raw(e20, psq0, Copy, bias=0.0, scale=1.0 / D)
            mu0f = consts.tile([1, TOK], f32, tag="ln_mu0f")
            act_raw(mu0f, psm0, Copy, bias=0.0, scale=1.0 / D)
            mu20 = consts.tile([1, TOK], f32, tag="ln_mu2")
            nc.scalar.activation(mu20, mu0f, Square)
            nc.vector.tensor_sub(e20, e20, mu20)
            std0b = consts.tile([1, TOK], bf16, tag="ln_std0")
            act_raw(std0b, e20, Sqrt, bias=eps_sb[:])

        # ---- y = res @ Wg2.T (bf16) + folds, token halves --------------
        acc = mid.tile([P, DK, TOK], bf16, tag="big16", name="acc")
        psq1 = [None, None]
        lnrows = [None, None]  # rstd1_b per half
        outh_t = [None, None]
        hss = [slice(0, HT), slice(HT, TOK)]

        def emit_y_half(h, hooks=None):
            hs = hss[h]
            psq1[h] = pss.tile([1, HT], f32, tag="psm", name=f"psq1{h}")
            lag = 1 if h == 0 else 0  # let mu0/nmu1 land before the first
            pend = []  # fold matmuls close a PSUM group
            hooks = hooks or {}
            for m in range(DK):
                pt = ps.tile([P, HT], f32, tag="pb", name=f"y{h}_{m}")
                for k in range(DK):
                    nc.tensor.matmul(
                        pt,
                        Wg2T_sb[:, k, m * P : (m + 1) * P],
                        resb[:, k, hs],
                        start=(k == 0),
                        stop=False,
                    )
                pend.append((m, pt))
                if len(pend) > lag:
                    _close_y(h, hs, *pend.pop(0))
                if m in hooks:
                    hooks[m]()
            while pend:
                _close_y(h, hs, *pend.pop(0))

        def _close_y(h, hs, m, pt):
            nc.tensor.matmul(
                pt, s2n_sb[:, m], mu0b[0:1, hs], start=False, stop=False
            )
            nc.tensor.matmul(
                pt, g1r_sb[:, m], nmu1[0:1, hs], start=False, stop=not has_cb
            )
            if has_cb:
                nc.tensor.matmul(
                    pt, cb_sb[:, m], std0b[0:1, hs], start=False, stop=True
                )
            nc.scalar.activation(acc[:, m, hs], pt, Copy)
            sq1 = ev.tile([P, HT], bf16, tag="sqb")
            if m >= DK - 2:  # keep the last squares off the DVE backlog
                nc.scalar.activation(sq1, acc[:, m, hs], Square)
            else:
                nc.vector.tensor_mul(sq1, acc[:, m, hs], acc[:, m, hs])
            nc.tensor.matmul(
                psq1[h],
                invg2_sb[:, 0, m : m + 1],
                sq1,
                start=(m == 0),
                stop=(m == DK - 1),
            )

        def emit_chain(h):
            """acc is centered, so LN1 is one Rsqrt off the variance PSUM
            plus one broadcast."""
            rstd1 = consts.tile([1, HT], bf16, tag="ln_rstd", name=f"rstd1{h}")
            act_raw(rstd1, psq1[h], Rsqrt, bias=eps_sb[:], scale=1.0 / D)
            lnrows[h] = bcast(rstd1, "rstd1", width=HT)
            outh_t[h] = mid.tile(
                [P, DK, HT], bf16, tag="outh", bufs=2, name=f"outh{h}"
            )

        def emit_wb_tile(h, m):
            """out[:, m] = acc*rstd1 (+ b1n when present)."""
            hs = hss[h]
            rstd1_b = lnrows[h]
            gp = m in (2, 5)  # gpsimd owns two tiles per half
            eng = nc.gpsimd if gp else nc.vector
            if has_b1n:
                t1 = ev.tile([P, HT], bf16, tag="ot", bufs=3)
                eng.tensor_mul(t1, acc[:, m, hs], rstd1_b)
                eng.tensor_scalar_add(outh_t[h][:, m], t1, b1n_sb[:, m : m + 1])
            else:
                eng.tensor_mul(outh_t[h][:, m], acc[:, m, hs], rstd1_b)
            if m % 2 == 1:  # flush every 2 tiles so the last DMA is small
                nc.sync.dma_start(
                    out=outT[h, :, m - 1 : m + 1],
                    in_=outh_t[h][:, m - 1 : m + 1],
                )

        # half 0 plain; half 1 interleaves half 0's chain (at m=1) and
        # writeback tiles (one per close from m=3) under its matmuls.
        emit_y_half(0)
        wb_state = {"n": 0}

        def _h1_hook_chain():
            emit_chain(0)

        def _h1_hook_wb():
            emit_wb_tile(0, wb_state["n"])
            wb_state["n"] += 1

        emit_y_half(
            1,
            hooks={
                1: _h1_hook_chain,
                3: _h1_hook_wb, 4: _h1_hook_wb, 5: _h1_hook_wb,
                6: _h1_hook_wb, 7: _h1_hook_wb,
            },
        )
        while wb_state["n"] < DK:
            _h1_hook_wb()
        emit_chain(1)
        for m in range(DK):
            emit_wb_tile(1, m)

    nc.finalize()
    return nc


def _get_nc(flags):
    key = ("nc",) + flags
    if key not in _cache:
        _cache[key] = _build_nc(*flags)
    return _cache[key]


def _pair_block_m(w):
    """[D, M] -> [P, M//P, KP, 2, P] m-major pair-blocked stationary.

    w[d, m] with d = (2*k + i)*P + p, m = mt*P + c lands at
    out[p, mt, k, i, c] so each [2, P] block is contiguous and each
    output-tile's weights are one contiguous DRAM run per partition.
    """
    Dd, M = w.shape
    return np.ascontiguousarray(
        w.reshape(Dd // (2 * P), 2, P, M // P, P).transpose(2, 3, 0, 1, 4)
    )


def _tblock(w):
    """[D, M] -> [P, D//P, M]: d = k*P + p lands at [p, k, :]."""
    Dd, M = w.shape
    return np.ascontiguousarray(w.reshape(Dd // P, P, M).transpose(1, 0, 2))


def _make_in_maps(inputs):
    import ml_dtypes

    bf = ml_dtypes.bfloat16
    f8 = ml_dtypes.float8_e4m3

    x = np.asarray(inputs["x"], dtype=np.float64)
    Wq = np.asarray(inputs["Wq"], np.float64)
    Wk = np.asarray(inputs["Wk"], np.float64)
    Wv = np.asarray(inputs["Wv"], np.float64)
    W1 = np.asarray(inputs["W1"], np.float64)
    W2 = np.asarray(inputs["W2"], np.float64)
    g0 = np.asarray(inputs["g0"], np.float64)
    b0 = np.asarray(inputs["b0"], np.float64)
    b1 = np.asarray(inputs["b1"], np.float64)
    b2 = np.asarray(inputs["b2"], np.float64)

    xf32 = x.astype(np.float32)
    x8 = xf32.astype(f8)
    xT8f = np.ascontiguousarray(xf32.T).astype(f8)

    Wf = W2 @ W1
    Wg = Wf + np.eye(D)
    g1f = np.asarray(inputs["g1"], np.float64)
    Wg2 = Wg * g0[None, :] * g1f[:, None]
    invg = 1.0 / g1f
    s2n = -Wg2.sum(axis=1)
    cb = (Wg @ b0 + W2 @ b1 + b2) * g1f
    # LN1 mean fold: mu1*D = wfold @ res (+ sum(cb/g1)*std0, folded into
    # the cb stationary below)
    wfold = Wg2.T @ invg + (np.dot(s2n, invg) / D)
    c2s = np.dot(cb, invg)
    cb2 = cb - (c2s / D) * g1f
    b1nf = np.asarray(inputs["b1n"], np.float64)
    lncon = np.stack(
        [
            wfold.astype(np.float32),
            (invg * invg).astype(np.float32),
            b1nf.astype(np.float32),
        ],
        axis=0,
    )  # [3, D]
    shared = {
        "B8d": _pair_block_m((WSCALE * (Wq.T @ Wk)).astype(np.float32).astype(f8)),
        "Wv8": _pair_block_m((WSCALE * Wv.T).astype(np.float32).astype(f8)),
        "Wg2T": _tblock(Wg2.T.astype(np.float32).astype(bf)),
        # [P, 3, DK]: row d = m*P + p of each vector at [p, i, m]
        "lncon": np.ascontiguousarray(
            lncon.reshape(3, DK, P).transpose(2, 0, 1)
        ),
        "s2nr": np.ascontiguousarray(
            s2n.astype(np.float32).reshape(1, DK, P)
        ).astype(bf),
        "g1r": np.ascontiguousarray(
            g1f.astype(np.float32).reshape(1, DK, P)
        ).astype(bf),
        "cbr": np.ascontiguousarray(
            cb2.astype(np.float32).reshape(1, DK, P)
        ).astype(bf),
        # scores stationary: [p, jt, k, i, m] = x[jt*P+m, (2k+i)*P+p]
        "xTg8": np.ascontiguousarray(
            xT8f.reshape(KP, 2, P, NJ, P).transpose(2, 3, 0, 1, 4)
        ),
        # attnx stationary: [p, mt, jp, i, m] = x[(2jp+i)*P+p, mt*P+m]
        "xg8": np.ascontiguousarray(
            x8.reshape(JP, 2, P, DK, P).transpose(2, 3, 0, 1, 4)
        ),
    }
    has_cb = bool(np.any(cb != 0.0))
    has_b1n = bool(np.any(b1nf != 0.0))
    bvf = np.asarray(inputs["bv"], np.float64)
    xTbv = (x + bvf[None, :]).T.astype(np.float32)
    xT = np.ascontiguousarray(xf32.T)
    in_maps = []
    for c in range(NCORES):
        m = dict(shared)
        m["xTb"] = _tblock(
            np.ascontiguousarray(xTbv[:, c * TOK : (c + 1) * TOK]).astype(bf)
        )
        # moving operand of xB: [p, k, i, t] = x[t, (2k+i)*P+p]
        xTl = np.ascontiguousarray(xT[:, c * TOK : (c + 1) * TOK]).astype(f8)
        m["xT8"] = np.ascontiguousarray(
            xTl.reshape(KP, 2, P, TOK).transpose(2, 0, 1, 3)
        )
        if not has_cb:
            del m["cbr"]
        in_maps.append(m)
    return in_maps, (has_cb, has_b1n)


def _assemble(res):
    out = np.empty((N, D), dtype=np.float32)
    for c in range(NCORES):
        # outT [2, P, DK, HT] bf16: out[h*HT+t, m*P+p] = arr[h, p, m, t]
        arr = np.asarray(res.results[c]["outT"], dtype=np.float32)
        out[c * TOK : (c + 1) * TOK, :] = arr.transpose(0, 3, 2, 1).reshape(TOK, D)
    return out


def kernel(**inputs):
    from concourse import bass_utils

    in_maps, flags = _make_in_maps(inputs)
    nc = _get_nc(flags)
    res = bass_utils.run_bass_kernel_spmd(
        nc, in_maps, core_ids=list(range(NCORES)), trace=False
    )
    return _assemble(res)


def run_traced(inputs):
    """Like kernel() but with NTFF tracing; returns (out, exec_time_ns, results)."""
    import hookshim

    hookshim.install()
    from concourse import bass_utils

    in_maps, flags = _make_in_maps(inputs)
    nc = _get_nc(flags)
    res = bass_utils.run_bass_kernel_spmd(
        nc, in_maps, core_ids=list(range(NCORES)), trace=True
    )
    return _assemble(res), res.exec_time_ns, res


# revision 10
# speedup vs baseline: 1.1917x; 1.0088x over previous
"""Distributed single-head transformer block on 8 TRN2 NeuronCores.

Collective-free restructuring. Algebraic folds done on the host
(weights only):
  - FFN has no activation between its two Linears, so it collapses to a
    single matrix Wf = W2@W1; the residual h folds in as Wg = Wf + I and
    LN0's gamma folds per-column: Wg2 = Wg * g0.
  - Q/K projections collapse into B = Wq.T @ Wk, so scores = x B x.T.
    Each core holds the FULL x (replicated at input-distribution time),
    so there is no K AllGather.
  - attn @ v = (P @ x) @ Wv.T + bv (softmax rows sum to 1), so there is
    no V AllGather either: P @ x uses the same resident full x.
  - LN0 folds via LN scale invariance: LN1(acc) == LN1(acc/rstd0), so
    the LN0 correction becomes acc2 = y + mu0*s2n (+ std0*cb with
    nonzero biases) -- 1-row bf16 matmuls accumulated INTO the y PSUM.
  - LN1's MEAN also folds into the y PSUM: mu1 = (wfold @ res)/D with
    wfold = Wg2.T(1/g1) + (sum(s2n/g1)/D)*ones is just another weight
    row, accumulated during the Wv phase. Subtracting g1 (x) mu1 as a
    fold matmul leaves acc CENTERED, so the LN1 chain is a single
    Rsqrt straight off the variance PSUM and the writeback is one
    multiply per tile (+ b1n scalar-add when present).

All large matmuls run in fp8 DoubleRow (2 contraction k-tiles per
instruction, 157 TF/s). The dual-fp8 ldweights ISA check requires each
(2,128) weight pair-block to be contiguous in SBUF, so the host
pre-permutes every stationary operand into [..., 2, 128]-blocked layout;
moving operands are written [..., 2, TOK]-blocked on chip.

Scheduling (v6):
  - sync HWDGE ring (in-order): xT8 -> B8d -> xTg8 in 8 chunks; scalar
    ring carries no early DMAs; background tensors ride gpsimd SWDGE
    emitted after the xB phase (gated by a copy of xB8) so they cannot
    steal front bandwidth.
  - y + LN1 + writeback split in token halves, half-outer; half 0's
    epilogue is emitted inside half 1's m-loop so the in-order engine
    queues interleave it under half 1's matmuls.
"""

import numpy as np

P = 128
D = 1024
N = 4096
NCORES = 8
TOK = N // NCORES  # 512 tokens per core
HT = TOK // 2  # 256-token halves for the y/LN1/writeback pipeline
DK = D // P  # 8 feature tiles
KP = DK // 2  # 4 feature pair-tiles
NJ = N // P  # 32 global token tiles
JP = NJ // 2  # 16 token pair-tiles
EPS = 1e-5
WSCALE = 16.0  # fp8 range scale on B and Wv
ASCALE = 32.0  # fp8 range scale on normalized attnx
SINV = 1.0 / 512.0  # 1/(WSCALE*sqrt(D)) exp logit scale; also 1/(WSCALE*ASCALE)

_cache = {}


def _build_nc(has_cb, has_b1n):
    import concourse.tile as tile
    from concourse import bacc, mybir
    from contextlib import ExitStack

    f32 = mybir.dt.float32
    bf16 = mybir.dt.bfloat16
    f8 = mybir.dt.float8e4
    Exp = mybir.ActivationFunctionType.Exp
    Sqrt = mybir.ActivationFunctionType.Sqrt
    Copy = mybir.ActivationFunctionType.Copy
    Square = mybir.ActivationFunctionType.Square
    DR = mybir.MatmulPerfMode.DoubleRow

    nc = bacc.Bacc("TRN2", target_bir_lowering=False, debug=False, num_devices=NCORES)

    # local shard (T-layout, pre-blocked): bf16 copy carries +bv prefolded
    # (residual only); fp8 copy is pure x for the score path
    xTb = nc.dram_tensor("xTb", [P, DK, TOK], bf16, kind="ExternalInput").ap()
    xT8 = nc.dram_tensor("xT8", [P, KP, 2, TOK], f8, kind="ExternalInput").ap()
    # full x, both layouts, fp8, host pre-permuted into pair-blocked form
    xTg8 = nc.dram_tensor("xTg8", [P, NJ, KP, 2, P], f8, kind="ExternalInput").ap()
    xg8 = nc.dram_tensor("xg8", [P, DK, JP, 2, P], f8, kind="ExternalInput").ap()
    # folded weights (pair-blocked fp8 stationaries)
    B8d = nc.dram_tensor("B8d", [P, DK, KP, 2, P], f8, kind="ExternalInput").ap()
    Wv8 = nc.dram_tensor("Wv8", [P, DK, KP, 2, P], f8, kind="ExternalInput").ap()
    Wg2T = nc.dram_tensor("Wg2T", [P, DK, D], bf16, kind="ExternalInput").ap()
    # [wfold; invg2; b1n] blocked [P, 3, DK] (per-partition columns)
    lncon = nc.dram_tensor("lncon", [P, 3, DK], f32, kind="ExternalInput").ap()
    # 1-row-blocked bf16 stationaries for the PE outer-product folds
    s2nr = nc.dram_tensor("s2nr", [1, DK, P], bf16, kind="ExternalInput").ap()
    g1r = nc.dram_tensor("g1r", [1, DK, P], bf16, kind="ExternalInput").ap()
    cbr = (
        nc.dram_tensor("cbr", [1, DK, P], bf16, kind="ExternalInput").ap()
        if has_cb
        else None
    )
    outT = nc.dram_tensor("outT", [2, P, DK, HT], bf16, kind="ExternalOutput").ap()

    with tile.TileContext(nc) as tc, ExitStack() as ctx:
        ctx.enter_context(
            nc.allow_low_precision("bf16 stat rows; LN-invariant rescale")
        )
        consts = ctx.enter_context(tc.tile_pool(name="consts", bufs=1))
        xin = ctx.enter_context(tc.tile_pool(name="xin", bufs=1))
        bigx = ctx.enter_context(tc.tile_pool(name="bigx", bufs=1))
        wp = ctx.enter_context(tc.tile_pool(name="wp", bufs=1))
        mid = ctx.enter_context(tc.tile_pool(name="mid", bufs=1))
        ev = ctx.enter_context(tc.tile_pool(name="ev", bufs=2))
        ps = ctx.enter_context(tc.tile_pool(name="ps", bufs=3, space="PSUM"))
        pss = ctx.enter_context(tc.tile_pool(name="pss", bufs=3, space="PSUM"))
        psb = ctx.enter_context(tc.tile_pool(name="psb", bufs=2, space="PSUM"))

        # ---- front-critical input DMAs, all on the sync HWDGE ring
        # (in-order): xT8 -> B8d (gates xB) -> xTg8 chunks (gate scores).
        xT8_sb = xin.tile([P, KP, 2, TOK], f8, tag="x8s")
        nc.scalar.dma_start(out=xT8_sb, in_=xT8)
        B8_sb = wp.tile([P, DK, KP, 2, P], f8)
        for c in range(4):
            nc.sync.dma_start(
                out=B8_sb[:, 2 * c : 2 * c + 2], in_=B8d[:, 2 * c : 2 * c + 2]
            )
        xTg_sb = bigx.tile([P, NJ, KP, 2, P], f8)
        for c in range(8):
            nc.sync.dma_start(
                out=xTg_sb[:, 4 * c : 4 * c + 4], in_=xTg8[:, 4 * c : 4 * c + 4]
            )

        # ---- constants -------------------------------------------------
        ones8 = consts.tile([P, 2, 16], f8)
        nc.vector.memset(ones8, 1.0)
        ones_b = consts.tile([P, 1], bf16)
        nc.vector.memset(ones_b, 1.0)
        onesr = consts.tile([1, P], bf16)
        nc.vector.memset(onesr, 1.0)
        eps_sb = consts.tile([1, 1], f32)
        nc.vector.memset(eps_sb, EPS)

        from concourse.bass import (
            AP,
            MemorySpace,
            assert_is_scalar,
            assert_partition_dims_match,
        )

        def act_raw(out, in_, func, bias=0.0, scale=1.0):
            eng = nc.scalar
            inputs = [eng.lower_ap(in_)]
            for arg in (bias, scale, 0.0):
                if isinstance(arg, AP):
                    assert_partition_dims_match(arg, in_)
                    assert_is_scalar(arg)
                    assert arg.space == MemorySpace.SBUF
                    inputs.append(eng.lower_ap(arg))
                else:
                    inputs.append(
                        mybir.ImmediateValue(dtype=mybir.dt.float32, value=arg)
                    )
            return eng.add_instruction(
                mybir.InstActivation(
                    name=eng.bass.get_next_instruction_name(),
                    func=func,
                    ins=inputs,
                    outs=[eng.lower_ap(out)],
                )
            )

        Rsqrt = mybir.ActivationFunctionType.Rsqrt
        Recip = mybir.ActivationFunctionType.Reciprocal

        _bc_n = [0]

        def bcast(row_b, tag, width=TOK):
            """[1, w] bf16 -> [P, w] bf16 broadcast via PE outer product."""
            _bc_n[0] += 1
            pt = psb.tile([P, width], f32, tag="bc", name=f"bc_{_bc_n[0]}")
            nc.tensor.matmul(pt, onesr, row_b, start=True, stop=True)
            sb = consts.tile(
                [P, width], bf16, name=f"bcs_{_bc_n[0]}", tag=f"bcs_{tag}"
            )
            nc.vector.tensor_copy(sb, pt)
            return sb

        # ---- xB = (16B) contract x (fp8 DoubleRow) ----------------------
        xB8_sb = mid.tile([P, KP, 2, TOK], f8)
        for m in range(DK):
            pt = ps.tile([P, TOK], f32, tag="pb")
            for k in range(KP):
                nc.tensor.matmul(
                    pt,
                    B8_sb[:, m, k],
                    xT8_sb[:, k],
                    start=(k == 0),
                    stop=(k == KP - 1),
                    perf_mode=DR,
                )
            nc.scalar.activation(xB8_sb[:, m // 2, m % 2, :], pt, Copy)

        # ---- background loads on gpsimd SWDGE, gated behind a tiny copy
        # of xB8 pair 0 so their transfers kick only once the front-
        # critical sync-ring traffic is nearly done. --------------------
        gate_t = ev.tile([P, 2, 1], bf16, tag="gate")
        nc.gpsimd.tensor_copy(gate_t, xB8_sb[:, KP - 1, :, 0:1])
        lncon_sb = consts.tile([P, 3, DK], f32)
        nc.gpsimd.dma_start(out=lncon_sb, in_=lncon)
        s2n_sb = consts.tile([1, DK, P], bf16)
        nc.gpsimd.dma_start(out=s2n_sb, in_=s2nr)
        g1r_sb = consts.tile([1, DK, P], bf16)
        nc.gpsimd.dma_start(out=g1r_sb, in_=g1r)
        if has_cb:
            cb_sb = consts.tile([1, DK, P], bf16)
            nc.gpsimd.dma_start(out=cb_sb, in_=cbr)
        xg_sb = bigx.tile([P, DK, JP, 2, P], f8)
        for c in range(8):
            nc.gpsimd.dma_start(out=xg_sb[:, c], in_=xg8[:, c])
        Wv8_sb = wp.tile([P, DK, KP, 2, P], f8)
        nc.gpsimd.dma_start(out=Wv8_sb, in_=Wv8)
        xTb_sb = xin.tile([P, DK, TOK], bf16)
        nc.gpsimd.dma_start(out=xTb_sb, in_=xTb)
        Wg2T_sb = wp.tile([P, DK, D], bf16)
        nc.gpsimd.dma_start(out=Wg2T_sb, in_=Wg2T)
        # wfold/invg2 bf16 per-partition stationaries; b1n f32 scalars
        wfold_sb = consts.tile([P, 1, DK], bf16)
        nc.vector.tensor_copy(wfold_sb, lncon_sb[:, 0:1])
        invg2_sb = consts.tile([P, 1, DK], bf16)
        nc.vector.tensor_copy(invg2_sb, lncon_sb[:, 1:2])
        b1n_sb = lncon_sb[:, 2]

        # ---- scores S^T + exp -> fp8 probs, denominator interleaved ----
        pT8 = mid.tile([P, JP, 2, TOK], f8, tag="big16")
        psd = pss.tile([1, TOK], f32, tag="psm")
        for j in range(NJ):
            pt = ps.tile([P, TOK], f32, tag="pb")
            for k in range(KP):
                nc.tensor.matmul(
                    pt,
                    xTg_sb[:, j, k],
                    xB8_sb[:, k],
                    start=(k == 0),
                    stop=(k == KP - 1),
                    perf_mode=DR,
                )
            nc.scalar.activation(pT8[:, j // 2, j % 2, :], pt, Exp, bias=0.0, scale=SINV)
            if j % 2 == 1:
                nc.tensor.matmul(
                    psd,
                    ones8[:, :, 0:1],
                    pT8[:, j // 2],
                    start=(j == 1),
                    stop=(j == NJ - 1),
                    perf_mode=DR,
                )
        rden32 = consts.tile([1, TOK], bf16)
        act_raw(rden32, psd, Recip, bias=0.0, scale=1.0 / ASCALE)

        # ---- attnx = P @ x (fp8 DoubleRow), normalized to fp8. The rden
        # broadcast matmul is issued AFTER m=0's matmuls so the PE queue
        # doesn't head-of-line block on the scalar reciprocal chain. ----
        attnx8 = xin.tile([P, KP, 2, TOK], f8, tag="x8s", name="attnx8")
        rden_b = None
        for m in range(DK):
            pt = ps.tile([P, TOK], f32, tag="pb")
            for j in range(JP):
                nc.tensor.matmul(
                    pt,
                    xg_sb[:, m, j],
                    pT8[:, j],
                    start=(j == 0),
                    stop=(j == JP - 1),
                    perf_mode=DR,
                )
            if m == 0:
                rden_b = bcast(rden32, "rden")
            nc.vector.tensor_mul(attnx8[:, m // 2, m % 2, :], pt, rden_b)

        # ---- attn_out = attnx @ (16Wv).T / 512 + (x + bv) = res.
        # psm0 (ones row) and psmW (wfold row) accumulate here: they
        # feed mu0 and the LN1 mean fold. --------------------------------
        resb = xin.tile([P, DK, TOK], bf16)
        psm0 = pss.tile([1, TOK], f32, tag="psm")
        psmW = pss.tile([1, TOK], f32, tag="psm")
        psq0 = pss.tile([1, TOK], f32, tag="psm") if has_cb else None
        for m in range(DK):
            pt = ps.tile([P, TOK], f32, tag="pb")
            for k in range(KP):
                nc.tensor.matmul(
                    pt,
                    Wv8_sb[:, m, k],
                    attnx8[:, k],
                    start=(k == 0),
                    stop=(k == KP - 1),
                    perf_mode=DR,
                )
            t1 = ev.tile([P, TOK], f32, tag="sq")
            nc.scalar.activation(t1, pt, Copy, bias=0.0, scale=SINV)
            nc.vector.tensor_add(resb[:, m, :], t1, xTb_sb[:, m, :])
            nc.tensor.matmul(
                psm0, ones_b, resb[:, m, :], start=(m == 0), stop=(m == DK - 1)
            )
            nc.tensor.matmul(
                psmW,
                wfold_sb[:, 0, m : m + 1],
                resb[:, m, :],
                start=(m == 0),
                stop=(m == DK - 1),
            )
            if has_cb:
                sq = ev.tile([P, TOK], bf16, tag="sqb")
                nc.scalar.activation(sq, resb[:, m, :], Square)
                nc.tensor.matmul(
                    psq0, ones_b, sq, start=(m == 0), stop=(m == DK - 1)
                )

        # ---- LN0 / LN1-mean scalars feeding the y-PSUM folds -----------
        mu0b = consts.tile([1, TOK], bf16, tag="ln_mu0")
        act_raw(mu0b, psm0, Copy, bias=0.0, scale=1.0 / D)
        nmu1 = consts.tile([1, TOK], bf16, tag="ln_nmu1")
        act_raw(nmu1, psmW, Copy, bias=0.0, scale=-1.0 / D)
        if has_cb:
            e20 = consts.tile([1, TOK], f32, tag="ln_e2")
            act_raw(e20, psq0, Copy, bias=0.0, scale=1.0 / D)
            mu0f = consts.tile([1, TOK], f32, tag="ln_mu0f")
            act_raw(mu0f, psm0, Copy, bias=0.0, scale=1.0 / D)
            mu20 = consts.tile([1, TOK], f32, tag="ln_mu2")
            nc.scalar.activation(mu20, mu0f, Square)
            nc.vector.tensor_sub(e20, e20, mu20)
            std0b = consts.tile([1, TOK], bf16, tag="ln_std0")
            act_raw(std0b, e20, Sqrt, bias=eps_sb[:])

        # ---- y = res @ Wg2.T (bf16) + folds, token halves --------------
        acc = mid.tile([P, DK, TOK], bf16, tag="big16", name="acc")
        psq1 = [None, None]
        lnrows = [None, None]  # rstd1_b per half
        outh_t = [None, None]
        hss = [slice(0, HT), slice(HT, TOK)]

        def emit_y_half(h, hooks=None):
            hs = hss[h]
            psq1[h] = pss.tile([1, HT], f32, tag="psm", name=f"psq1{h}")
            lag = 1 if h == 0 else 0  # let mu0/nmu1 land before the first
            pend = []  # fold matmuls close a PSUM group
            hooks = hooks or {}
            for m in range(DK):
                pt = ps.tile([P, HT], f32, tag="pb", name=f"y{h}_{m}")
                for k in range(DK):
                    nc.tensor.matmul(
                        pt,
                        Wg2T_sb[:, k, m * P : (m + 1) * P],
                        resb[:, k, hs],
                        start=(k == 0),
                        stop=False,
                    )
                pend.append((m, pt))
                if len(pend) > lag:
                    _close_y(h, hs, *pend.pop(0))
                if m in hooks:
                    hooks[m]()
            while pend:
                _close_y(h, hs, *pend.pop(0))

        def _close_y(h, hs, m, pt):
            nc.tensor.matmul(
                pt, s2n_sb[:, m], mu0b[0:1, hs], start=False, stop=False
            )
            nc.tensor.matmul(
                pt, g1r_sb[:, m], nmu1[0:1, hs], start=False, stop=not has_cb
            )
            if has_cb:
                nc.tensor.matmul(
                    pt, cb_sb[:, m], std0b[0:1, hs], start=False, stop=True
                )
            sq1 = ev.tile([P, HT], bf16, tag="sqb")
            nc.scalar.activation(sq1, pt, Square)
            nc.scalar.activation(acc[:, m, hs], pt, Copy)
            nc.tensor.matmul(
                psq1[h],
                invg2_sb[:, 0, m : m + 1],
                sq1,
                start=(m == 0),
                stop=(m == DK - 1),
            )

        def emit_chain(h):
            """acc is centered, so LN1 is one Rsqrt off the variance PSUM
            plus one broadcast."""
            rstd1 = consts.tile([1, HT], bf16, tag="ln_rstd", name=f"rstd1{h}")
            act_raw(rstd1, psq1[h], Rsqrt, bias=eps_sb[:], scale=1.0 / D)
            lnrows[h] = bcast(rstd1, "rstd1", width=HT)
            outh_t[h] = mid.tile(
                [P, DK, HT], bf16, tag="outh", bufs=2, name=f"outh{h}"
            )

        def emit_wb_tile(h, m):
            """out[:, m] = acc*rstd1 (+ b1n when present)."""
            hs = hss[h]
            rstd1_b = lnrows[h]
            gp = m in (2, 5)  # gpsimd owns two tiles per half
            eng = nc.gpsimd if gp else nc.vector
            if has_b1n:
                t1 = ev.tile([P, HT], bf16, tag="ot", bufs=3)
                eng.tensor_mul(t1, acc[:, m, hs], rstd1_b)
                eng.tensor_scalar_add(outh_t[h][:, m], t1, b1n_sb[:, m : m + 1])
            else:
                eng.tensor_mul(outh_t[h][:, m], acc[:, m, hs], rstd1_b)
            if m % 2 == 1:  # flush every 2 tiles so the last DMA is small
                ring = nc.sync if m % 4 == 1 else nc.scalar
                ring.dma_start(
                    out=outT[h, :, m - 1 : m + 1],
                    in_=outh_t[h][:, m - 1 : m + 1],
                )

        # half 0 plain; half 1 interleaves half 0's chain (at m=1) and
        # writeback tiles (one per close from m=3) under its matmuls.
        emit_y_half(0)
        wb_state = {"n": 0}

        def _h1_hook_chain():
            emit_chain(0)

        def _h1_hook_wb():
            emit_wb_tile(0, wb_state["n"])
            wb_state["n"] += 1

        emit_y_half(
            1,
            hooks={
                1: _h1_hook_chain,
                3: _h1_hook_wb, 4: _h1_hook_wb, 5: _h1_hook_wb,
                6: _h1_hook_wb, 7: _h1_hook_wb,
            },
        )
        while wb_state["n"] < DK:
            _h1_hook_wb()
        emit_chain(1)
        for m in range(DK):
            emit_wb_tile(1, m)

    nc.finalize()
    return nc


def _get_nc(flags):
    key = ("nc",) + flags
    if key not in _cache:
        _cache[key] = _build_nc(*flags)
    return _cache[key]


def _pair_block_m(w):
    """[D, M] -> [P, M//P, KP, 2, P] m-major pair-blocked stationary.

    w[d, m] with d = (2*k + i)*P + p, m = mt*P + c lands at
    out[p, mt, k, i, c] so each [2, P] block is contiguous and each
    output-tile's weights are one contiguous DRAM run per partition.
    """
    Dd, M = w.shape
    return np.ascontiguousarray(
        w.reshape(Dd // (2 * P), 2, P, M // P, P).transpose(2, 3, 0, 1, 4)
    )


def _tblock(w):
    """[D, M] -> [P, D//P, M]: d = k*P + p lands at [p, k, :]."""
    Dd, M = w.shape
    return np.ascontiguousarray(w.reshape(Dd // P, P, M).transpose(1, 0, 2))


def _make_in_maps(inputs):
    import ml_dtypes

    bf = ml_dtypes.bfloat16
    f8 = ml_dtypes.float8_e4m3

    x = np.asarray(inputs["x"], dtype=np.float64)
    Wq = np.asarray(inputs["Wq"], np.float64)
    Wk = np.asarray(inputs["Wk"], np.float64)
    Wv = np.asarray(inputs["Wv"], np.float64)
    W1 = np.asarray(inputs["W1"], np.float64)
    W2 = np.asarray(inputs["W2"], np.float64)
    g0 = np.asarray(inputs["g0"], np.float64)
    b0 = np.asarray(inputs["b0"], np.float64)
    b1 = np.asarray(inputs["b1"], np.float64)
    b2 = np.asarray(inputs["b2"], np.float64)

    xf32 = x.astype(np.float32)
    x8 = xf32.astype(f8)
    xT8f = np.ascontiguousarray(xf32.T).astype(f8)

    Wf = W2 @ W1
    Wg = Wf + np.eye(D)
    g1f = np.asarray(inputs["g1"], np.float64)
    Wg2 = Wg * g0[None, :] * g1f[:, None]
    invg = 1.0 / g1f
    s2n = -Wg2.sum(axis=1)
    cb = (Wg @ b0 + W2 @ b1 + b2) * g1f
    # LN1 mean fold: mu1*D = wfold @ res (+ sum(cb/g1)*std0, folded into
    # the cb stationary below)
    wfold = Wg2.T @ invg + (np.dot(s2n, invg) / D)
    c2s = np.dot(cb, invg)
    cb2 = cb - (c2s / D) * g1f
    b1nf = np.asarray(inputs["b1n"], np.float64)
    lncon = np.stack(
        [
            wfold.astype(np.float32),
            (invg * invg).astype(np.float32),
            b1nf.astype(np.float32),
        ],
        axis=0,
    )  # [3, D]
    shared = {
        "B8d": _pair_block_m((WSCALE * (Wq.T @ Wk)).astype(np.float32).astype(f8)),
        "Wv8": _pair_block_m((WSCALE * Wv.T).astype(np.float32).astype(f8)),
        "Wg2T": _tblock(Wg2.T.astype(np.float32).astype(bf)),
        # [P, 3, DK]: row d = m*P + p of each vector at [p, i, m]
        "lncon": np.ascontiguousarray(
            lncon.reshape(3, DK, P).transpose(2, 0, 1)
        ),
        "s2nr": np.ascontiguousarray(
            s2n.astype(np.float32).reshape(1, DK, P)
        ).astype(bf),
        "g1r": np.ascontiguousarray(
            g1f.astype(np.float32).reshape(1, DK, P)
        ).astype(bf),
        "cbr": np.ascontiguousarray(
            cb2.astype(np.float32).reshape(1, DK, P)
        ).astype(bf),
        # scores stationary: [p, jt, k, i, m] = x[jt*P+m, (2k+i)*P+p]
        "xTg8": np.ascontiguousarray(
            xT8f.reshape(KP, 2, P, NJ, P).transpose(2, 3, 0, 1, 4)
        ),
        # attnx stationary: [p, mt, jp, i, m] = x[(2jp+i)*P+p, mt*P+m]
        "xg8": np.ascontiguousarray(
            x8.reshape(JP, 2, P, DK, P).transpose(2, 3, 0, 1, 4)
        ),
    }
    has_cb = bool(np.any(cb != 0.0))
    has_b1n = bool(np.any(b1nf != 0.0))
    bvf = np.asarray(inputs["bv"], np.float64)
    xTbv = (x + bvf[None, :]).T.astype(np.float32)
    xT = np.ascontiguousarray(xf32.T)
    in_maps = []
    for c in range(NCORES):
        m = dict(shared)
        m["xTb"] = _tblock(
            np.ascontiguousarray(xTbv[:, c * TOK : (c + 1) * TOK]).astype(bf)
        )
        # moving operand of xB: [p, k, i, t] = x[t, (2k+i)*P+p]
        xTl = np.ascontiguousarray(xT[:, c * TOK : (c + 1) * TOK]).astype(f8)
        m["xT8"] = np.ascontiguousarray(
            xTl.reshape(KP, 2, P, TOK).transpose(2, 0, 1, 3)
        )
        if not has_cb:
            del m["cbr"]
        in_maps.append(m)
    return in_maps, (has_cb, has_b1n)


def _assemble(res):
    out = np.empty((N, D), dtype=np.float32)
    for c in range(NCORES):
        # outT [2, P, DK, HT] bf16: out[h*HT+t, m*P+p] = arr[h, p, m, t]
        arr = np.asarray(res.results[c]["outT"], dtype=np.float32)
        out[c * TOK : (c + 1) * TOK, :] = arr.transpose(0, 3, 2, 1).reshape(TOK, D)
    return out


def kernel(**inputs):
    from concourse import bass_utils

    in_maps, flags = _make_in_maps(inputs)
    nc = _get_nc(flags)
    res = bass_utils.run_bass_kernel_spmd(
        nc, in_maps, core_ids=list(range(NCORES)), trace=False
    )
    return _assemble(res)


def run_traced(inputs):
    """Like kernel() but with NTFF tracing; returns (out, exec_time_ns, results)."""
    import hookshim

    hookshim.install()
    from concourse import bass_utils

    in_maps, flags = _make_in_maps(inputs)
    nc = _get_nc(flags)
    res = bass_utils.run_bass_kernel_spmd(
        nc, in_maps, core_ids=list(range(NCORES)), trace=True
    )
    return _assemble(res), res.exec_time_ns, res


# revision 12
# speedup vs baseline: 1.2425x; 1.0426x over previous
"""Distributed single-head transformer block on 8 TRN2 NeuronCores.

Collective-free restructuring. Algebraic folds done on the host
(weights only):
  - FFN has no activation between its two Linears, so it collapses to a
    single matrix Wf = W2@W1; the residual h folds in as Wg = Wf + I and
    LN0's gamma folds per-column: Wg2 = Wg * g0.
  - Q/K projections collapse into B = Wq.T @ Wk, so scores = x B x.T.
    Each core holds the FULL x (replicated at input-distribution time),
    so there is no K AllGather.
  - attn @ v = (P @ x) @ Wv.T + bv (softmax rows sum to 1), so there is
    no V AllGather either: P @ x uses the same resident full x.
  - LN0 folds via LN scale invariance: LN1(acc) == LN1(acc/rstd0), so
    the LN0 correction becomes acc2 = y + mu0*s2n (+ std0*cb with
    nonzero biases) -- 1-row bf16 matmuls accumulated INTO the y PSUM.
  - LN1's MEAN also folds into the y PSUM: mu1 = (wfold @ res)/D with
    wfold = Wg2.T(1/g1) + (sum(s2n/g1)/D)*ones is just another weight
    row, accumulated during the Wv phase. Subtracting g1 (x) mu1 as a
    fold matmul leaves acc CENTERED, so the LN1 chain is a single
    Rsqrt straight off the variance PSUM and the writeback is one
    multiply per tile (+ b1n scalar-add when present).

All large matmuls run in fp8 DoubleRow (2 contraction k-tiles per
instruction, 157 TF/s). The dual-fp8 ldweights ISA check requires each
(2,128) weight pair-block to be contiguous in SBUF, so the host
pre-permutes every stationary operand into [..., 2, 128]-blocked layout;
moving operands are written [..., 2, TOK]-blocked on chip.

Scheduling (v6):
  - sync HWDGE ring (in-order): xT8 -> B8d -> xTg8 in 8 chunks; scalar
    ring carries no early DMAs; background tensors ride gpsimd SWDGE
    emitted after the xB phase (gated by a copy of xB8) so they cannot
    steal front bandwidth.
  - y + LN1 + writeback split in token halves, half-outer; half 0's
    epilogue is emitted inside half 1's m-loop so the in-order engine
    queues interleave it under half 1's matmuls.
"""

import numpy as np

P = 128
D = 1024
N = 4096
NCORES = 8
TOK = N // NCORES  # 512 tokens per core
HT = TOK // 2  # 256-token halves for the y/LN1/writeback pipeline
DK = D // P  # 8 feature tiles
KP = DK // 2  # 4 feature pair-tiles
NJ = N // P  # 32 global token tiles
JP = NJ // 2  # 16 token pair-tiles
EPS = 1e-5
WSCALE = 16.0  # fp8 range scale on B and Wv
ASCALE = 32.0  # fp8 range scale on normalized attnx
SINV = 1.0 / 512.0  # 1/(WSCALE*sqrt(D)) exp logit scale; also 1/(WSCALE*ASCALE)
WO_SCALE = 512.0  # fp8 range scale on the off-diagonal FFN fold Wo
RSCALE = 16.0  # fp8 range scale on res
YSC = WO_SCALE * RSCALE  # y PSUM accumulates at this scale

_cache = {}


def _build_nc(has_cb, has_b1n):
    import concourse.tile as tile
    from concourse import bacc, mybir
    from contextlib import ExitStack

    f32 = mybir.dt.float32
    bf16 = mybir.dt.bfloat16
    f8 = mybir.dt.float8e4
    Exp = mybir.ActivationFunctionType.Exp
    Sqrt = mybir.ActivationFunctionType.Sqrt
    Copy = mybir.ActivationFunctionType.Copy
    Square = mybir.ActivationFunctionType.Square
    DR = mybir.MatmulPerfMode.DoubleRow

    nc = bacc.Bacc("TRN2", target_bir_lowering=False, debug=False, num_devices=NCORES)

    # local shard (T-layout, pre-blocked): bf16 copy carries +bv prefolded
    # (residual only); fp8 copy is pure x for the score path
    xTb = nc.dram_tensor("xTb", [P, DK, TOK], bf16, kind="ExternalInput").ap()
    xT8 = nc.dram_tensor("xT8", [P, KP, 2, TOK], f8, kind="ExternalInput").ap()
    # full x, both layouts, fp8, host pre-permuted into pair-blocked form
    xTg8 = nc.dram_tensor("xTg8", [P, NJ, KP, 2, P], f8, kind="ExternalInput").ap()
    xg8 = nc.dram_tensor("xg8", [P, DK, JP, 2, P], f8, kind="ExternalInput").ap()
    # folded weights (pair-blocked fp8 stationaries)
    B8d = nc.dram_tensor("B8d", [P, DK, KP, 2, P], f8, kind="ExternalInput").ap()
    Wv8 = nc.dram_tensor("Wv8", [P, DK, KP, 2, P], f8, kind="ExternalInput").ap()
    Wo8 = nc.dram_tensor("Wo8", [P, DK, KP, 2, P], f8, kind="ExternalInput").ap()
    dgd = nc.dram_tensor("dgd", [P, DK, P], bf16, kind="ExternalInput").ap()
    # [wfold; invg2; b1n] blocked [P, 3, DK] (per-partition columns)
    lncon = nc.dram_tensor("lncon", [P, 3, DK], f32, kind="ExternalInput").ap()
    # 1-row-blocked bf16 stationaries for the PE outer-product folds
    s2nr = nc.dram_tensor("s2nr", [1, DK, P], bf16, kind="ExternalInput").ap()
    g1r = nc.dram_tensor("g1r", [1, DK, P], bf16, kind="ExternalInput").ap()
    cbr = (
        nc.dram_tensor("cbr", [1, DK, P], bf16, kind="ExternalInput").ap()
        if has_cb
        else None
    )
    outT = nc.dram_tensor("outT", [2, P, DK, HT], bf16, kind="ExternalOutput").ap()

    with tile.TileContext(nc) as tc, ExitStack() as ctx:
        ctx.enter_context(
            nc.allow_low_precision("bf16 stat rows; LN-invariant rescale")
        )
        consts = ctx.enter_context(tc.tile_pool(name="consts", bufs=1))
        xin = ctx.enter_context(tc.tile_pool(name="xin", bufs=1))
        bigx = ctx.enter_context(tc.tile_pool(name="bigx", bufs=1))
        wp = ctx.enter_context(tc.tile_pool(name="wp", bufs=1))
        mid = ctx.enter_context(tc.tile_pool(name="mid", bufs=1))
        ev = ctx.enter_context(tc.tile_pool(name="ev", bufs=2))
        ps = ctx.enter_context(tc.tile_pool(name="ps", bufs=3, space="PSUM"))
        pss = ctx.enter_context(tc.tile_pool(name="pss", bufs=3, space="PSUM"))
        psb = ctx.enter_context(tc.tile_pool(name="psb", bufs=2, space="PSUM"))

        # ---- front-critical input DMAs, all on the sync HWDGE ring
        # (in-order): xT8 -> B8d (gates xB) -> xTg8 chunks (gate scores).
        xT8_sb = xin.tile([P, KP, 2, TOK], f8, tag="x8s")
        nc.sync.dma_start(out=xT8_sb, in_=xT8)
        B8_sb = wp.tile([P, DK, KP, 2, P], f8)
        for c in range(4):
            ring = nc.scalar if c % 2 == 0 else nc.sync
            ring.dma_start(
                out=B8_sb[:, 2 * c : 2 * c + 2], in_=B8d[:, 2 * c : 2 * c + 2]
            )
        xTg_sb = bigx.tile([P, NJ, KP, 2, P], f8)
        for c in range(8):
            nc.sync.dma_start(
                out=xTg_sb[:, 4 * c : 4 * c + 4], in_=xTg8[:, 4 * c : 4 * c + 4]
            )

        # ---- constants -------------------------------------------------
        ones8 = consts.tile([P, 2, 16], f8)
        nc.vector.memset(ones8, 1.0)
        ones_b = consts.tile([P, 1], bf16)
        nc.vector.memset(ones_b, 1.0)
        onesr = consts.tile([1, P], bf16)
        nc.vector.memset(onesr, 1.0)
        eps_sb = consts.tile([1, 1], f32)
        nc.vector.memset(eps_sb, EPS)

        from concourse.bass import (
            AP,
            MemorySpace,
            assert_is_scalar,
            assert_partition_dims_match,
        )

        def act_raw(out, in_, func, bias=0.0, scale=1.0):
            eng = nc.scalar
            inputs = [eng.lower_ap(in_)]
            for arg in (bias, scale, 0.0):
                if isinstance(arg, AP):
                    assert_partition_dims_match(arg, in_)
                    assert_is_scalar(arg)
                    assert arg.space == MemorySpace.SBUF
                    inputs.append(eng.lower_ap(arg))
                else:
                    inputs.append(
                        mybir.ImmediateValue(dtype=mybir.dt.float32, value=arg)
                    )
            return eng.add_instruction(
                mybir.InstActivation(
                    name=eng.bass.get_next_instruction_name(),
                    func=func,
                    ins=inputs,
                    outs=[eng.lower_ap(out)],
                )
            )

        Rsqrt = mybir.ActivationFunctionType.Rsqrt
        Recip = mybir.ActivationFunctionType.Reciprocal

        _bc_n = [0]

        def bcast(row_b, tag, width=TOK):
            """[1, w] bf16 -> [P, w] bf16 broadcast via PE outer product."""
            _bc_n[0] += 1
            pt = psb.tile([P, width], f32, tag="bc", name=f"bc_{_bc_n[0]}")
            nc.tensor.matmul(pt, onesr, row_b, start=True, stop=True)
            sb = consts.tile(
                [P, width], bf16, name=f"bcs_{_bc_n[0]}", tag=f"bcs_{tag}"
            )
            nc.vector.tensor_copy(sb, pt)
            return sb

        # ---- xB = (16B) contract x (fp8 DoubleRow) ----------------------
        xB8_sb = mid.tile([P, KP, 2, TOK], f8)
        for m in range(DK):
            pt = ps.tile([P, TOK], f32, tag="pb")
            for k in range(KP):
                nc.tensor.matmul(
                    pt,
                    B8_sb[:, m, k],
                    xT8_sb[:, k],
                    start=(k == 0),
                    stop=(k == KP - 1),
                    perf_mode=DR,
                )
            nc.scalar.activation(xB8_sb[:, m // 2, m % 2, :], pt, Copy)

        # ---- background loads on gpsimd SWDGE, gated behind a tiny copy
        # of xB8 pair 0 so their transfers kick only once the front-
        # critical sync-ring traffic is nearly done. --------------------
        gate_t = ev.tile([P, 2, 1], bf16, tag="gate")
        nc.gpsimd.tensor_copy(gate_t, xB8_sb[:, KP - 1, :, 0:1])
        lncon_sb = consts.tile([P, 3, DK], f32)
        nc.gpsimd.dma_start(out=lncon_sb, in_=lncon)
        s2n_sb = consts.tile([1, DK, P], bf16)
        nc.gpsimd.dma_start(out=s2n_sb, in_=s2nr)
        g1r_sb = consts.tile([1, DK, P], bf16)
        nc.gpsimd.dma_start(out=g1r_sb, in_=g1r)
        if has_cb:
            cb_sb = consts.tile([1, DK, P], bf16)
            nc.gpsimd.dma_start(out=cb_sb, in_=cbr)
        xg_sb = bigx.tile([P, DK, JP, 2, P], f8)
        for c in range(8):
            nc.gpsimd.dma_start(out=xg_sb[:, c], in_=xg8[:, c])
        Wv8_sb = wp.tile([P, DK, KP, 2, P], f8)
        nc.gpsimd.dma_start(out=Wv8_sb, in_=Wv8)
        xTb_sb = xin.tile([P, DK, TOK], bf16)
        nc.gpsimd.dma_start(out=xTb_sb, in_=xTb)
        Wo8_sb = wp.tile([P, DK, KP, 2, P], f8)
        nc.gpsimd.dma_start(out=Wo8_sb, in_=Wo8)
        dg_sb = wp.tile([P, DK, P], bf16)
        nc.gpsimd.dma_start(out=dg_sb, in_=dgd)
        # wfold/invg2 bf16 per-partition stationaries; b1n f32 scalars
        wfold_sb = consts.tile([P, 1, DK], bf16)
        nc.vector.tensor_copy(wfold_sb, lncon_sb[:, 0:1])
        invg2_sb = consts.tile([P, 1, DK], bf16)
        nc.vector.tensor_copy(invg2_sb, lncon_sb[:, 1:2])
        b1n_sb = lncon_sb[:, 2]

        # ---- scores S^T + exp -> fp8 probs, denominator interleaved ----
        pT8 = mid.tile([P, JP, 2, TOK], f8, tag="big16")
        psd = pss.tile([1, TOK], f32, tag="psm")
        for j in range(NJ):
            pt = ps.tile([P, TOK], f32, tag="pb")
            for k in range(KP):
                nc.tensor.matmul(
                    pt,
                    xTg_sb[:, j, k],
                    xB8_sb[:, k],
                    start=(k == 0),
                    stop=(k == KP - 1),
                    perf_mode=DR,
                )
            nc.scalar.activation(pT8[:, j // 2, j % 2, :], pt, Exp, bias=0.0, scale=SINV)
            if j % 2 == 1:
                nc.tensor.matmul(
                    psd,
                    ones8[:, :, 0:1],
                    pT8[:, j // 2],
                    start=(j == 1),
                    stop=(j == NJ - 1),
                    perf_mode=DR,
                )
        rden32 = consts.tile([1, TOK], bf16)
        act_raw(rden32, psd, Recip, bias=0.0, scale=1.0 / ASCALE)

        # ---- attnx = P @ x (fp8 DoubleRow), normalized to fp8. The rden
        # broadcast matmul is issued AFTER m=0's matmuls so the PE queue
        # doesn't head-of-line block on the scalar reciprocal chain. ----
        attnx8 = xin.tile([P, KP, 2, TOK], f8, tag="x8s", name="attnx8")
        rden_b = None
        for m in range(DK):
            pt = ps.tile([P, TOK], f32, tag="pb")
            for j in range(JP):
                nc.tensor.matmul(
                    pt,
                    xg_sb[:, m, j],
                    pT8[:, j],
                    start=(j == 0),
                    stop=(j == JP - 1),
                    perf_mode=DR,
                )
            if m == 0:
                rden_b = bcast(rden32, "rden")
            nc.vector.tensor_mul(attnx8[:, m // 2, m % 2, :], pt, rden_b)

        # ---- attn_out = attnx @ (16Wv).T / 512 + (x + bv) = res.
        # psm0 (ones row) and psmW (wfold row) accumulate here: they
        # feed mu0 and the LN1 mean fold. --------------------------------
        resb = xin.tile([P, DK, TOK], bf16)
        res8 = mid.tile([P, KP, 2, TOK], f8, tag="res8")
        psm0 = pss.tile([1, TOK], f32, tag="psm")
        psmW = pss.tile([1, TOK], f32, tag="psm")
        psq0 = pss.tile([1, TOK], f32, tag="psm") if has_cb else None
        for m in range(DK):
            pt = ps.tile([P, TOK], f32, tag="pb")
            for k in range(KP):
                nc.tensor.matmul(
                    pt,
                    Wv8_sb[:, m, k],
                    attnx8[:, k],
                    start=(k == 0),
                    stop=(k == KP - 1),
                    perf_mode=DR,
                )
            t1 = ev.tile([P, TOK], f32, tag="sq")
            nc.scalar.activation(t1, pt, Copy, bias=0.0, scale=SINV)
            nc.vector.tensor_add(resb[:, m, :], t1, xTb_sb[:, m, :])
            if m % 2 == 0:
                nc.scalar.activation(
                    res8[:, m // 2, m % 2, :], resb[:, m, :], Copy, scale=RSCALE
                )
            else:
                nc.vector.tensor_scalar_mul(
                    res8[:, m // 2, m % 2, :], resb[:, m, :], float(RSCALE)
                )
            nc.tensor.matmul(
                psm0, ones_b, resb[:, m, :], start=(m == 0), stop=(m == DK - 1)
            )
            nc.tensor.matmul(
                psmW,
                wfold_sb[:, 0, m : m + 1],
                resb[:, m, :],
                start=(m == 0),
                stop=(m == DK - 1),
            )
            if has_cb:
                sq = ev.tile([P, TOK], bf16, tag="sqb")
                nc.scalar.activation(sq, resb[:, m, :], Square)
                nc.tensor.matmul(
                    psq0, ones_b, sq, start=(m == 0), stop=(m == DK - 1)
                )

        # ---- LN0 / LN1-mean scalars feeding the y-PSUM folds -----------
        mu0b = consts.tile([1, TOK], bf16, tag="ln_mu0")
        act_raw(mu0b, psm0, Copy, bias=0.0, scale=YSC / D)
        nmu1 = consts.tile([1, TOK], bf16, tag="ln_nmu1")
        act_raw(nmu1, psmW, Copy, bias=0.0, scale=-YSC / D)
        if has_cb:
            e20 = consts.tile([1, TOK], f32, tag="ln_e2")
            act_raw(e20, psq0, Copy, bias=0.0, scale=1.0 / D)
            mu0f = consts.tile([1, TOK], f32, tag="ln_mu0f")
            act_raw(mu0f, psm0, Copy, bias=0.0, scale=1.0 / D)
            mu20 = consts.tile([1, TOK], f32, tag="ln_mu2")
            nc.scalar.activation(mu20, mu0f, Square)
            nc.vector.tensor_sub(e20, e20, mu20)
            std0b = consts.tile([1, TOK], bf16, tag="ln_std0")
            act_raw(std0b, e20, Sqrt, bias=eps_sb[:])
            nc.vector.tensor_scalar_mul(std0b, std0b, float(YSC))

        # ---- y = res @ Wg2.T (bf16) + folds, token halves --------------
        acc = mid.tile([P, DK, TOK], bf16, tag="big16", name="acc")
        psq1 = [None, None]
        lnrows = [None, None]  # rstd1_b per half
        outh_t = [None, None]
        hss = [slice(0, HT), slice(HT, TOK)]

        def emit_y_half(h, hooks=None):
            hs = hss[h]
            psq1[h] = pss.tile([1, HT], f32, tag="psm", name=f"psq1{h}")
            lag = 1 if h == 0 else 0  # let mu0/nmu1 land before the first
            pend = []  # fold matmuls close a PSUM group
            hooks = hooks or {}
            for m in range(DK):
                pt = ps.tile([P, HT], f32, tag="pb", name=f"y{h}_{m}")
                for k in range(KP):
                    nc.tensor.matmul(
                        pt,
                        Wo8_sb[:, m, k],
                        res8[:, k, :, hs],
                        start=(k == 0),
                        stop=False,
                        perf_mode=DR,
                    )
                nc.tensor.matmul(
                    pt, dg_sb[:, m], resb[:, m, hs], start=False, stop=False
                )
                pend.append((m, pt))
                if len(pend) > lag:
                    _close_y(h, hs, *pend.pop(0))
                if m in hooks:
                    hooks[m]()
            while pend:
                _close_y(h, hs, *pend.pop(0))

        def _close_y(h, hs, m, pt):
            nc.tensor.matmul(
                pt, s2n_sb[:, m], mu0b[0:1, hs], start=False, stop=False
            )
            nc.tensor.matmul(
                pt, g1r_sb[:, m], nmu1[0:1, hs], start=False, stop=not has_cb
            )
            if has_cb:
                nc.tensor.matmul(
                    pt, cb_sb[:, m], std0b[0:1, hs], start=False, stop=True
                )
            sq1 = ev.tile([P, HT], bf16, tag="sqb")
            nc.scalar.activation(sq1, pt, Square, bias=0.0, scale=1.0 / YSC)
            nc.scalar.activation(acc[:, m, hs], pt, Copy, bias=0.0, scale=1.0 / YSC)
            nc.tensor.matmul(
                psq1[h],
                invg2_sb[:, 0, m : m + 1],
                sq1,
                start=(m == 0),
                stop=(m == DK - 1),
            )

        def emit_chain(h):
            """acc is centered, so LN1 is one Rsqrt off the variance PSUM
            plus one broadcast."""
            rstd1 = consts.tile([1, HT], bf16, tag="ln_rstd", name=f"rstd1{h}")
            act_raw(rstd1, psq1[h], Rsqrt, bias=eps_sb[:], scale=1.0 / D)
            lnrows[h] = bcast(rstd1, "rstd1", width=HT)
            outh_t[h] = mid.tile(
                [P, DK, HT], bf16, tag="outh", bufs=2, name=f"outh{h}"
            )

        def emit_wb_tile(h, m):
            """out[:, m] = acc*rstd1 (+ b1n when present)."""
            hs = hss[h]
            rstd1_b = lnrows[h]
            gp = m in (2, 5)  # gpsimd owns two tiles per half
            eng = nc.gpsimd if gp else nc.vector
            if has_b1n:
                t1 = ev.tile([P, HT], bf16, tag="ot", bufs=3)
                eng.tensor_mul(t1, acc[:, m, hs], rstd1_b)
                eng.tensor_scalar_add(outh_t[h][:, m], t1, b1n_sb[:, m : m + 1])
            else:
                eng.tensor_mul(outh_t[h][:, m], acc[:, m, hs], rstd1_b)
            if m % 2 == 1:  # flush every 2 tiles so the last DMA is small
                ring = nc.sync if m % 4 == 1 else nc.scalar
                ring.dma_start(
                    out=outT[h, :, m - 1 : m + 1],
                    in_=outh_t[h][:, m - 1 : m + 1],
                )

        # half 0 plain; half 1 interleaves half 0's chain (at m=1) and
        # writeback tiles (one per close from m=3) under its matmuls.
        emit_y_half(0)
        wb_state = {"n": 0}

        def _h1_hook_chain():
            emit_chain(0)

        def _h1_hook_wb():
            emit_wb_tile(0, wb_state["n"])
            wb_state["n"] += 1

        emit_y_half(
            1,
            hooks={
                1: _h1_hook_chain,
                3: _h1_hook_wb, 4: _h1_hook_wb, 5: _h1_hook_wb,
                6: _h1_hook_wb, 7: _h1_hook_wb,
            },
        )
        while wb_state["n"] < DK:
            _h1_hook_wb()
        emit_chain(1)
        for m in range(DK):
            emit_wb_tile(1, m)

    nc.finalize()
    return nc


def _get_nc(flags):
    key = ("nc",) + flags
    if key not in _cache:
        _cache[key] = _build_nc(*flags)
    return _cache[key]


def _pair_block_m(w):
    """[D, M] -> [P, M//P, KP, 2, P] m-major pair-blocked stationary.

    w[d, m] with d = (2*k + i)*P + p, m = mt*P + c lands at
    out[p, mt, k, i, c] so each [2, P] block is contiguous and each
    output-tile's weights are one contiguous DRAM run per partition.
    """
    Dd, M = w.shape
    return np.ascontiguousarray(
        w.reshape(Dd // (2 * P), 2, P, M // P, P).transpose(2, 3, 0, 1, 4)
    )


def _tblock(w):
    """[D, M] -> [P, D//P, M]: d = k*P + p lands at [p, k, :]."""
    Dd, M = w.shape
    return np.ascontiguousarray(w.reshape(Dd // P, P, M).transpose(1, 0, 2))


def _dg_block(dg):
    """diag(Wg2) -> [P, DK, P] bf16 per-m-tile diagonal stationaries,
    scaled so the y PSUM accumulates at x YSC."""
    import ml_dtypes

    out = np.zeros((P, DK, P), dtype=np.float32)
    for m in range(DK):
        out[np.arange(P), m, np.arange(P)] = YSC * dg[m * P : m * P + P]
    return out.astype(ml_dtypes.bfloat16)


def _make_in_maps(inputs):
    import ml_dtypes

    bf = ml_dtypes.bfloat16
    f8 = ml_dtypes.float8_e4m3

    x = np.asarray(inputs["x"], dtype=np.float64)
    Wq = np.asarray(inputs["Wq"], np.float64)
    Wk = np.asarray(inputs["Wk"], np.float64)
    Wv = np.asarray(inputs["Wv"], np.float64)
    W1 = np.asarray(inputs["W1"], np.float64)
    W2 = np.asarray(inputs["W2"], np.float64)
    g0 = np.asarray(inputs["g0"], np.float64)
    b0 = np.asarray(inputs["b0"], np.float64)
    b1 = np.asarray(inputs["b1"], np.float64)
    b2 = np.asarray(inputs["b2"], np.float64)

    xf32 = x.astype(np.float32)
    x8 = xf32.astype(f8)
    xT8f = np.ascontiguousarray(xf32.T).astype(f8)

    Wf = W2 @ W1
    Wg = Wf + np.eye(D)
    g1f = np.asarray(inputs["g1"], np.float64)
    Wg2 = Wg * g0[None, :] * g1f[:, None]
    invg = 1.0 / g1f
    dg = np.diag(Wg2).copy()
    Wo = Wg2 - np.diag(dg)
    s2n = -Wg2.sum(axis=1)
    cb = (Wg @ b0 + W2 @ b1 + b2) * g1f
    # LN1 mean fold: mu1*D = wfold @ res (+ sum(cb/g1)*std0, folded into
    # the cb stationary below)
    wfold = Wg2.T @ invg + (np.dot(s2n, invg) / D)
    c2s = np.dot(cb, invg)
    cb2 = cb - (c2s / D) * g1f
    b1nf = np.asarray(inputs["b1n"], np.float64)
    lncon = np.stack(
        [
            wfold.astype(np.float32),
            (invg * invg).astype(np.float32),
            b1nf.astype(np.float32),
        ],
        axis=0,
    )  # [3, D]
    shared = {
        "B8d": _pair_block_m((WSCALE * (Wq.T @ Wk)).astype(np.float32).astype(f8)),
        "Wv8": _pair_block_m((WSCALE * Wv.T).astype(np.float32).astype(f8)),
        "Wo8": _pair_block_m(
            (WO_SCALE * Wo.T).astype(np.float32).astype(f8)
        ),
        "dgd": _dg_block(dg),
        # [P, 3, DK]: row d = m*P + p of each vector at [p, i, m]
        "lncon": np.ascontiguousarray(
            lncon.reshape(3, DK, P).transpose(2, 0, 1)
        ),
        "s2nr": np.ascontiguousarray(
            s2n.astype(np.float32).reshape(1, DK, P)
        ).astype(bf),
        "g1r": np.ascontiguousarray(
            g1f.astype(np.float32).reshape(1, DK, P)
        ).astype(bf),
        "cbr": np.ascontiguousarray(
            cb2.astype(np.float32).reshape(1, DK, P)
        ).astype(bf),
        # scores stationary: [p, jt, k, i, m] = x[jt*P+m, (2k+i)*P+p]
        "xTg8": np.ascontiguousarray(
            xT8f.reshape(KP, 2, P, NJ, P).transpose(2, 3, 0, 1, 4)
        ),
        # attnx stationary: [p, mt, jp, i, m] = x[(2jp+i)*P+p, mt*P+m]
        "xg8": np.ascontiguousarray(
            x8.reshape(JP, 2, P, DK, P).transpose(2, 3, 0, 1, 4)
        ),
    }
    has_cb = bool(np.any(cb != 0.0))
    has_b1n = bool(np.any(b1nf != 0.0))
    bvf = np.asarray(inputs["bv"], np.float64)
    xTbv = (x + bvf[None, :]).T.astype(np.float32)
    xT = np.ascontiguousarray(xf32.T)
    in_maps = []
    for c in range(NCORES):
        m = dict(shared)
        m["xTb"] = _tblock(
            np.ascontiguousarray(xTbv[:, c * TOK : (c + 1) * TOK]).astype(bf)
        )
        # moving operand of xB: [p, k, i, t] = x[t, (2k+i)*P+p]
        xTl = np.ascontiguousarray(xT[:, c * TOK : (c + 1) * TOK]).astype(f8)
        m["xT8"] = np.ascontiguousarray(
            xTl.reshape(KP, 2, P, TOK).transpose(2, 0, 1, 3)
        )
        if not has_cb:
            del m["cbr"]
        in_maps.append(m)
    return in_maps, (has_cb, has_b1n)


def _assemble(res):
    out = np.empty((N, D), dtype=np.float32)
    for c in range(NCORES):
        # outT [2, P, DK, HT] bf16: out[h*HT+t, m*P+p] = arr[h, p, m, t]
        arr = np.asarray(res.results[c]["outT"], dtype=np.float32)
        out[c * TOK : (c + 1) * TOK, :] = arr.transpose(0, 3, 2, 1).reshape(TOK, D)
    return out


def kernel(**inputs):
    from concourse import bass_utils

    in_maps, flags = _make_in_maps(inputs)
    nc = _get_nc(flags)
    res = bass_utils.run_bass_kernel_spmd(
        nc, in_maps, core_ids=list(range(NCORES)), trace=False
    )
    return _assemble(res)


def run_traced(inputs):
    """Like kernel() but with NTFF tracing; returns (out, exec_time_ns, results)."""
    import hookshim

    hookshim.install()
    from concourse import bass_utils

    in_maps, flags = _make_in_maps(inputs)
    nc = _get_nc(flags)
    res = bass_utils.run_bass_kernel_spmd(
        nc, in_maps, core_ids=list(range(NCORES)), trace=True
    )
    return _assemble(res), res.exec_time_ns, res
